# revision 1
# baseline (speedup 1.0000x reference)
"""3-layer GAT (nn_GATModel) on 8 Trainium2 NeuronCores — self-contained kernel.

kernel(**inputs) takes the FULL inputs (x [50000,64], edge_index [2,800000],
edge_attr [800000,8], batch, params pytree) and returns the FULL [50000,1] output.

Sharding strategy (edge/graph partitioning per the hint, specialized):
  - Destination nodes are range-partitioned across the 8 cores (6250 real + 22 pad
    rows -> 6272 = 49*128 rows per core). Each core owns the complete segment
    softmax + aggregation for its destinations, so no per-edge collectives are
    needed; the only collective is one AllGather of the projected node table per
    layer.
  - Per layer, each core projects its own activations into table rows
    [xs | a_s | a_d | pad] (a_s/a_d/a_e are attention terms pre-folded into
    per-node scalars: att_src/att_dst/att_edge contract with W_src/W_dst/W_edge
    on the host into [d_in, H] matrices). AllGather -> full table on every core.
  - Edges are laid out as ELL slots per destination: 8 slots per src-half
    (src < 25088 vs >=, so gather indices fit int16 for the fast dma_gather
    ucode), padded with a dummy node whose a_s = -1e6 (=> exp weight exactly 0);
    extra edges go to one-hot overflow chunks appended to the same gather call.
  - Per (window of 128 dsts, half): ONE dma_gather fetches all slot rows
    (768B/512B rows), spread over 4 SWDGE queues for ~3x descriptor throughput.
    Attention scores are computed in place; one matmul per 128-slot chunk
    aggregates exp-weighted messages AND softmax denominators into PSUM
    (lhsT = static slot->dst one-hot; overflow chunks use is_equal-built one-hots).
  - Window epilogue: divide by denominator (+1e-16), add bias, PE-transpose to
    build the next layer's lhsT blocks; layer 3 applies the folded linear head
    (ll/fl collapse to [64,1]) and sigmoid, emitting the per-core output shard.
  - The segment softmax skips the max-subtraction (exactly equivalent
    mathematically; scores are O(10) so exp is safe in fp32), with the dummy
    slots underflowing to 0.
"""
import sys

for _p in ('/opt/trn_rl_repo', '/root/.axon_site/_ro/trn_rl_repo'):
    if _p not in sys.path:
        sys.path.insert(0, _p)

import numpy as np

import concourse.bass as bass
import concourse.bacc as bacc
import concourse.tile as tile
import concourse.mybir as mybir
from concourse import bass_utils
from concourse.library_config import mlp

P = 128
ELL = 8           # slots per dst per src-half
GD = 16           # dsts per chunk (= 128/ELL)
NCORES = 8

N_NODES = 50000
N_EDGES = 800000
RPC = N_NODES // NCORES          # 6250 real nodes per core
NW = (RPC + P - 1) // P          # 49 windows
NPC = NW * P                     # 6272 padded rows per core
N_PAD = NPC * NCORES             # 50176
HALF = NPC * (NCORES // 2)       # 25088 src split
DUMMY = [RPC, (NCORES // 2) * NPC + RPC]
WT = [192, 192, 128]             # table row stride per layer (256B multiples)
WX = [128, 128, 64]              # xs width per layer (H*C)
HA = [2, 2, 1]                   # heads per layer
WB = [132, 132, 66]              # written cols per layer


def _remap(ids):
    c = ids // RPC
    return c * NPC + (ids - c * RPC)


def _preprocess(edge_index):
    src = _remap(np.asarray(edge_index[0], np.int64))
    dst = _remap(np.asarray(edge_index[1], np.int64))
    core_of = dst // NPC
    halfb = (src >= HALF).astype(np.int64)

    novf = np.zeros((NCORES, NW, 2), np.int64)
    per_core_data = []
    for c in range(NCORES):
        m = core_of == c
        s_c, d_c, h_c, eid_c = src[m], dst[m] - c * NPC, halfb[m], np.nonzero(m)[0]
        w_c, dl_c = d_c // P, d_c % P
        order = np.lexsort((dl_c, h_c, w_c))
        s_c, dl_c, h_c, w_c, eid_c = (a[order] for a in (s_c, dl_c, h_c, w_c, eid_c))
        key = (w_c * 2 + h_c) * P + dl_c
        change = np.r_[True, key[1:] != key[:-1]]
        startidx = np.nonzero(change)[0]
        runlen = np.diff(np.r_[startidx, len(key)])
        rank = np.arange(len(key)) - np.repeat(startidx, runlen)
        ell_mask = rank < ELL
        per_core_data.append((s_c, dl_c, h_c, w_c, eid_c, rank, ell_mask))
        ov = ~ell_mask
        for w in range(NW):
            for hb in range(2):
                novf[c, w, hb] = np.sum(ov & (w_c == w) & (h_c == hb))

    ovch = np.zeros((NW, 2), np.int64)
    for w in range(NW):
        for hb in range(2):
            ovch[w, hb] = int((novf[:, w, hb].max() + P - 1) // P)

    S_wh = np.zeros((NW, 2), np.int64)
    offs = np.zeros((NW, 2), np.int64)
    ovf_col0 = np.zeros((NW, 2), np.int64)
    tot = 0
    ovftot = 0
    for w in range(NW):
        for hb in range(2):
            S_wh[w, hb] = ELL * P + ovch[w, hb] * P
            offs[w, hb] = tot
            tot += int(S_wh[w, hb])
            ovf_col0[w, hb] = ovftot
            ovftot += int(ovch[w, hb])
    plan = dict(S_wh=S_wh, offs=offs, ovch=ovch, ovf_col0=ovf_col0,
                tot_slots=tot, tot_ovf_chunks=ovftot)

    percore = []
    for c in range(NCORES):
        s_c, dl_c, h_c, w_c, eid_c, rank, em = per_core_data[c]
        slot_src = np.zeros(tot, np.int64)
        slot_eid = np.full(tot, -1, np.int64)
        ovf_dstloc = np.zeros((P, max(ovftot, 1)), np.float32)
        for w in range(NW):
            for hb in range(2):
                o = int(offs[w, hb])
                slot_src[o:o + int(S_wh[w, hb])] = DUMMY[hb]
        chunk = dl_c // GD
        pin = (dl_c % GD) * ELL + rank
        slot_pos = offs[w_c, h_c] + chunk * P + pin
        sp = slot_pos[em]
        slot_src[sp] = s_c[em]
        slot_eid[sp] = eid_c[em]
        ovm = ~em
        if np.any(ovm):
            for w in range(NW):
                for hb in range(2):
                    mm = ovm & (w_c == w) & (h_c == hb)
                    k = int(mm.sum())
                    if k == 0:
                        continue
                    base = int(offs[w, hb]) + ELL * P
                    pos = base + np.arange(k)
                    slot_src[pos] = s_c[mm]
                    slot_eid[pos] = eid_c[mm]
                    for j in range(int(ovch[w, hb])):
                        colv = np.zeros(P, np.float32)
                        lo, hi = j * P, min((j + 1) * P, k)
                        if lo < k:
                            colv[0:hi - lo] = dl_c[mm][lo:hi].astype(np.float32)
                        ovf_dstloc[:, int(ovf_col0[w, hb]) + j] = colv
        idx_rel = slot_src.copy()
        for w in range(NW):
            for hb in range(2):
                o = int(offs[w, hb])
                if hb == 1:
                    idx_rel[o:o + int(S_wh[w, hb])] -= HALF
        assert idx_rel.min() >= 0 and idx_rel.max() < 32768
        cols = tot // 16
        idx16 = np.zeros((16, cols), np.int16)
        pos = 0
        for w in range(NW):
            for hb in range(2):
                blk = idx_rel[int(offs[w, hb]):int(offs[w, hb]) + int(S_wh[w, hb])]
                n16 = len(blk) // 16
                idx16[:, pos:pos + n16] = blk.reshape(n16, 16).T
                pos += n16
        idx16 = np.tile(idx16, (8, 1))
        percore.append(dict(idx16=idx16, slot_eid=slot_eid, ovf_dstloc=ovf_dstloc))
    return plan, percore


def _fold_params(params):
    def g(p, k):
        return np.asarray(p[k], np.float32)
    out = {}
    for li in range(3):
        p = params[li]
        H, C = HA[li], 64
        W_src, W_dst, W_edge = g(p, 'W_src'), g(p, 'W_dst'), g(p, 'W_edge')
        a_s, a_d, a_e = g(p, 'att_src')[0], g(p, 'att_dst')[0], g(p, 'att_edge')[0]
        d_in = W_src.shape[0]
        As = np.zeros((d_in, H), np.float32)
        Ad = np.zeros((d_in, H), np.float32)
        V = np.zeros((8, H), np.float32)
        for h in range(H):
            As[:, h] = W_src[:, h * C:(h + 1) * C] @ a_s[h]
            Ad[:, h] = W_dst[:, h * C:(h + 1) * C] @ a_d[h]
            V[:, h] = W_edge[:, h * C:(h + 1) * C] @ a_e[h]
        out[f'rhsW{li}'] = np.concatenate([W_src, As, Ad], axis=1)
        out[f'V{li}'] = V
        out[f'bias{li}'] = np.broadcast_to(g(p, 'bias'), (P, H * C)).copy()
    hp = params[3]
    llW, llb = np.asarray(hp['ll_W'], np.float32), np.asarray(hp['ll_b'], np.float32)
    flW, flb = np.asarray(hp['fl_W'], np.float32), np.asarray(hp['fl_b'], np.float32)
    out['Whead'] = (llW @ flW).astype(np.float32)
    out['bhead'] = float(llb @ flW[:, 0] + flb[0])
    B128 = np.zeros((ELL, P, P), np.float32)
    for c in range(ELL):
        for p_ in range(P):
            B128[c, p_, GD * c + p_ // ELL] = 1.0
    out['B128'] = B128
    out['E128'] = np.ascontiguousarray(B128.transpose(0, 2, 1))
    out['iota'] = np.broadcast_to(np.arange(P, dtype=np.float32), (P, P)).copy()
    out['ident'] = np.eye(P, dtype=np.float32)
    return out


def _build_nc(plan):
    nc = bacc.Bacc("TRN2", target_bir_lowering=False, debug=False,
                   num_devices=NCORES, num_swdge_queues=4)
    dt = mybir.dt.float32
    S_wh, offs, ovch, ovf_col0 = plan['S_wh'], plan['offs'], plan['ovch'], plan['ovf_col0']
    tot, totov = plan['tot_slots'], max(plan['tot_ovf_chunks'], 1)

    xT0 = nc.dram_tensor("xT0", [64, NPC], dt, kind="ExternalInput")
    idx16 = nc.dram_tensor("idx16", [P, tot // 16], mybir.dt.int16, kind="ExternalInput")
    eaT = nc.dram_tensor("eaT", [8, tot], dt, kind="ExternalInput")
    ovfdl = nc.dram_tensor("ovfdl", [P, totov], dt, kind="ExternalInput")
    prm = {}
    for li in range(3):
        d_in = 64 if li == 0 else 128
        prm[f'rhsW{li}'] = nc.dram_tensor(f"rhsW{li}", [d_in, WB[li]], dt, kind="ExternalInput")
        prm[f'V{li}'] = nc.dram_tensor(f"V{li}", [8, HA[li]], dt, kind="ExternalInput")
        prm[f'bias{li}'] = nc.dram_tensor(f"bias{li}", [P, WX[li]], dt, kind="ExternalInput")
    prm['Whead'] = nc.dram_tensor("Whead", [64, 1], dt, kind="ExternalInput")
    prm['bheadneg'] = nc.dram_tensor("bheadneg", [1, 1], dt, kind="ExternalInput")
    prm['B128'] = nc.dram_tensor("B128", [ELL, P, P], dt, kind="ExternalInput")
    prm['padmask'] = nc.dram_tensor("padmask", [P, 2], dt, kind="ExternalInput")
    prm['E128'] = nc.dram_tensor("E128", [ELL, P, P], dt, kind="ExternalInput")
    prm['iota'] = nc.dram_tensor("iota", [P, P], dt, kind="ExternalInput")
    prm['ident'] = nc.dram_tensor("ident", [P, P], dt, kind="ExternalInput")
    y_out = nc.dram_tensor("y", [1, NPC], dt, kind="ExternalOutput")

    xT = [None,
          nc.dram_tensor("xT1", [P, NPC], dt, kind="Internal"),
          nc.dram_tensor("xT2", [P, NPC], dt, kind="Internal")]
    Tin, Tfull = [], []
    for li in range(3):
        Tin.append(nc.dram_tensor(f"Tin{li}", [NPC, WT[li]], dt, kind="Internal"))
        Tfull.append(nc.dram_tensor(f"Tfull{li}", [N_PAD, WT[li]], dt,
                                    kind="Internal", addr_space="Shared"))

    rr = [0]
    with tile.TileContext(nc) as tc:
        with (
            tc.tile_pool(name="const", bufs=1) as cp,
            tc.tile_pool(name="sbuf", bufs=3) as sb,
            tc.tile_pool(name="gpool", bufs=3) as gp,
            tc.tile_pool(name="psum", bufs=2, space="PSUM") as ps,
            tc.tile_pool(name="psmisc", bufs=2, space="PSUM") as pm,
        ):
            nc.gpsimd.load_library(mlp)
            c_t = {}
            for name, shape in [('padmask', [P, 2]), ('iota', [P, P]),
                                ('ident', [P, P]), ('Whead', [64, 1]), ('bheadneg', [1, 1])]:
                c_t[name] = cp.tile(shape, dt, name=name, tag=name)
                nc.sync.dma_start(out=c_t[name][:], in_=prm[name][:])
            for c in range(ELL):
                for nm in ('B128', 'E128'):
                    c_t[f'{nm}_{c}'] = cp.tile([P, P], dt, name=f'{nm}_{c}', tag=f'{nm}_{c}')
                    nc.sync.dma_start(out=c_t[f'{nm}_{c}'][:], in_=prm[nm][c])
            for li in range(3):
                d_in = 64 if li == 0 else 128
                c_t[f'rhsW{li}'] = cp.tile([d_in, WB[li]], dt, name=f"rhsW{li}", tag=f"rhsW{li}")
                nc.sync.dma_start(out=c_t[f'rhsW{li}'][:], in_=prm[f'rhsW{li}'][:])
                c_t[f'V{li}'] = cp.tile([8, HA[li]], dt, name=f"V{li}", tag=f"V{li}")
                nc.sync.dma_start(out=c_t[f'V{li}'][:], in_=prm[f'V{li}'][:])
                c_t[f'bias{li}'] = cp.tile([P, WX[li]], dt, name=f"bias{li}", tag=f"bias{li}")
                nc.sync.dma_start(out=c_t[f'bias{li}'][:], in_=prm[f'bias{li}'][:])

            for li in range(3):
                WTl, WXl, HAl, WBl = WT[li], WX[li], HA[li], WB[li]
                d_in = 64 if li == 0 else 128
                for j in range(NW):
                    lhs = sb.tile([d_in, P], dt, tag="tb_lhs")
                    src_ap = xT0[:, j * P:(j + 1) * P] if li == 0 else xT[li][:, j * P:(j + 1) * P]
                    nc.sync.dma_start(out=lhs[:], in_=src_ap)
                    pt = pm.tile([P, WBl], dt, space="PSUM", tag="tb_ps")
                    nc.tensor.matmul(pt[:], lhsT=lhs[:], rhs=c_t[f'rhsW{li}'][:],
                                     start=True, stop=True)
                    ts = sb.tile([P, WBl], dt, tag="tb_sb")
                    nc.vector.tensor_copy(ts[:], pt[:])
                    if j == NW - 1:
                        nc.vector.tensor_tensor(
                            out=ts[:, WXl:WXl + HAl], in0=ts[:, WXl:WXl + HAl],
                            in1=c_t['padmask'][:, 0:HAl], op=mybir.AluOpType.add)
                    nc.sync.dma_start(out=Tin[li][j * P:(j + 1) * P, 0:WBl], in_=ts[:])
                nc.gpsimd.collective_compute(
                    "AllGather", mybir.AluOpType.bypass,
                    replica_groups=[list(range(NCORES))],
                    ins=[Tin[li][:, :]], outs=[Tfull[li][:, :]],
                )
                for w in range(NW):
                    adw = sb.tile([P, HAl], dt, tag="adw")
                    nc.sync.dma_start(
                        out=adw[:],
                        in_=Tin[li][w * P:(w + 1) * P, WXl + HAl:WXl + 2 * HAl],
                    )
                    pw = ps.tile([P, WXl + HAl], dt, space="PSUM", tag="pwin")
                    half_data = []
                    for hb in range(2):
                        S = int(S_wh[w, hb])
                        K = S // P
                        o = int(offs[w, hb])
                        idx_t = sb.tile([P, S // 16], mybir.dt.int16, tag="idx")
                        nc.sync.dma_start(out=idx_t[:], in_=idx16[:, o // 16:(o + S) // 16])
                        ea_t = sb.tile([8, S], dt, tag="ea")
                        nc.sync.dma_start(out=ea_t[:], in_=eaT[:, o:o + S])
                        g = gp.tile([P, K, WTl], dt, tag=f"g{hb}")
                        base = Tfull[li][HALF:, 0:WTl] if hb else Tfull[li][:, 0:WTl]
                        nc.gpsimd.dma_gather(
                            out_ap=g[:], in_ap=base, idxs_ap=idx_t[:],
                            num_idxs=S, num_idxs_reg=S, elem_size=WTl, elem_step=WTl,
                            single_packet=False, queue_num=rr[0] % 4,
                        )
                        rr[0] += 1
                        pae = ps.tile([P, K * HAl], dt, space="PSUM", tag=f"pae{hb}", bufs=1)
                        ovP = []
                        for k in range(K):
                            nc.tensor.matmul(
                                pae[:, k * HAl:(k + 1) * HAl],
                                lhsT=ea_t[:, k * P:(k + 1) * P], rhs=c_t[f'V{li}'][:],
                                start=True, stop=False)
                            if k < ELL:
                                nc.tensor.matmul(
                                    pae[:, k * HAl:(k + 1) * HAl],
                                    lhsT=c_t[f'E128_{k}'][:], rhs=adw[:, :],
                                    start=False, stop=True)
                            else:
                                oc = int(ovf_col0[w, hb]) + (k - ELL)
                                p_sb = sb.tile([P, P], dt, tag=f"povf{hb}_{k - ELL}")
                                dl = sb.tile([P, 1], dt, tag=f"dl{hb}_{k - ELL}")
                                nc.sync.dma_start(out=dl[:], in_=ovfdl[:, oc:oc + 1])
                                nc.vector.tensor_tensor(
                                    out=p_sb[:], in0=dl[:].to_broadcast([P, P]),
                                    in1=c_t['iota'][:], op=mybir.AluOpType.is_equal)
                                ptp = pm.tile([P, P], dt, space="PSUM", tag="ptrans", bufs=1)
                                nc.tensor.transpose(out=ptp[:], in_=p_sb[:], identity=c_t['ident'][:])
                                pT_sb = sb.tile([P, P], dt, tag=f"pT{hb}_{k - ELL}")
                                nc.vector.tensor_copy(pT_sb[:], ptp[:])
                                nc.tensor.matmul(
                                    pae[:, k * HAl:(k + 1) * HAl],
                                    lhsT=pT_sb[:], rhs=adw[:, :],
                                    start=False, stop=True)
                                ovP.append(p_sb)
                        t_a = sb.tile([P, K, HAl], dt, tag=f"ta{hb}")
                        nc.vector.tensor_tensor(
                            out=t_a[:], in0=g[:, :, WXl:WXl + HAl],
                            in1=pae[:].rearrange("p (k h) -> p k h", h=HAl),
                            op=mybir.AluOpType.add)
                        t2 = sb.tile([P, K, HAl], dt, tag=f"t2{hb}")
                        nc.vector.tensor_scalar(out=t2[:], in0=t_a[:], scalar1=0.2,
                                                scalar2=None, op0=mybir.AluOpType.mult)
                        lr = sb.tile([P, K, HAl], dt, tag=f"lr{hb}")
                        nc.vector.tensor_tensor(out=lr[:], in0=t_a[:], in1=t2[:],
                                                op=mybir.AluOpType.max)
                        nc.scalar.activation(g[:, :, WXl:WXl + HAl], lr[:],
                                             mybir.ActivationFunctionType.Exp)
                        for h in range(HAl):
                            nc.vector.tensor_tensor(
                                out=g[:, :, h * 64:(h + 1) * 64],
                                in0=g[:, :, h * 64:(h + 1) * 64],
                                in1=g[:, :, WXl + h:WXl + h + 1].to_broadcast([P, K, 64]),
                                op=mybir.AluOpType.mult)
                        half_data.append((g, ovP, K))
                    ktot = half_data[0][2] + half_data[1][2]
                    kc = 0
                    for hb in range(2):
                        g, ovP, K = half_data[hb]
                        for k in range(K):
                            lh = c_t[f'B128_{k}'][:] if k < ELL else ovP[k - ELL][:]
                            nc.tensor.matmul(
                                pw[:, :], lhsT=lh, rhs=g[:, k:k + 1, 0:WXl + HAl],
                                start=(kc == 0), stop=(kc == ktot - 1),
                                skip_group_check=True)
                            kc += 1
                    dn = sb.tile([P, HAl], dt, tag="dn")
                    nc.vector.tensor_scalar(out=dn[:], in0=pw[:, WXl:WXl + HAl],
                                            scalar1=1e-16, scalar2=None,
                                            op0=mybir.AluOpType.add)
                    rc = sb.tile([P, HAl], dt, tag="rc")
                    nc.vector.reciprocal(rc[:], dn[:])
                    o_sb = sb.tile([P, WXl], dt, tag="osb")
                    for h in range(HAl):
                        nc.vector.tensor_scalar(
                            out=o_sb[:, h * 64:(h + 1) * 64], in0=pw[:, h * 64:(h + 1) * 64],
                            scalar1=rc[:, h:h + 1], scalar2=None, op0=mybir.AluOpType.mult)
                    nc.vector.tensor_tensor(out=o_sb[:], in0=o_sb[:], in1=c_t[f'bias{li}'][:],
                                            op=mybir.AluOpType.add)
                    ptp2 = pm.tile([WXl, P], dt, space="PSUM", tag="ptrans", bufs=1)
                    nc.tensor.transpose(out=ptp2[:], in_=o_sb[:], identity=c_t['ident'][:])
                    if li < 2:
                        xts = sb.tile([WXl, P], dt, tag="xts")
                        nc.vector.tensor_copy(xts[:], ptp2[:])
                        nc.sync.dma_start(out=xT[li + 1][:, w * P:(w + 1) * P], in_=xts[:])
                    else:
                        o3t = sb.tile([64, P], dt, tag="o3t")
                        nc.vector.tensor_copy(o3t[:], ptp2[:])
                        py = pm.tile([1, P], dt, space="PSUM", tag="py", bufs=1)
                        nc.tensor.matmul(py[:], lhsT=c_t['Whead'][:], rhs=o3t[:],
                                         start=True, stop=True)
                        es = sb.tile([1, P], dt, tag="es")
                        nc.scalar.activation(es[:], py[:], mybir.ActivationFunctionType.Exp,
                                             scale=-1.0, bias=c_t['bheadneg'][0:1, 0:1])
                        e1 = sb.tile([1, P], dt, tag="e1")
                        nc.vector.tensor_scalar(out=e1[:], in0=es[:], scalar1=1.0,
                                                scalar2=None, op0=mybir.AluOpType.add)
                        ys = sb.tile([1, P], dt, tag="ys")
                        nc.vector.reciprocal(ys[:], e1[:])
                        nc.sync.dma_start(out=y_out[:, w * P:(w + 1) * P], in_=ys[:])
    nc.compile()
    return nc


def _make_inmaps(plan, percore, x, edge_attr, pf):
    x = np.asarray(x, np.float32)
    ea = np.asarray(edge_attr, np.float32)
    in_maps = []
    pmsk = np.zeros((P, 2), np.float32)
    if RPC % P:
        pmsk[RPC % P:, :] = -1e6
    for c in range(NCORES):
        d = percore[c]
        xT0 = np.zeros((64, NPC), np.float32)
        xs = x[c * RPC:(c + 1) * RPC]
        xT0[:, :xs.shape[0]] = xs.T
        eaT = np.zeros((8, plan['tot_slots']), np.float32)
        m = d['slot_eid'] >= 0
        eaT[:, m] = ea[d['slot_eid'][m]].T
        im = dict(xT0=xT0, idx16=d['idx16'], eaT=eaT, ovfdl=d['ovf_dstloc'],
                  padmask=pmsk)
        for li in range(3):
            im[f'rhsW{li}'] = pf[f'rhsW{li}']
            im[f'V{li}'] = pf[f'V{li}']
            im[f'bias{li}'] = pf[f'bias{li}']
        im['Whead'] = pf['Whead']
        im['bheadneg'] = np.array([[-pf['bhead']]], np.float32)
        for k in ('B128', 'E128', 'iota', 'ident'):
            im[k] = pf[k]
        in_maps.append(im)
    return in_maps


_CACHE = {}


def kernel(x, edge_index, edge_attr, batch=None, params=None):
    ei = np.asarray(edge_index).astype(np.int64)
    key = hash(ei.tobytes())
    if key in _CACHE:
        plan, percore, nc = _CACHE[key]
    else:
        plan, percore = _preprocess(ei)
        nc = _build_nc(plan)
        _CACHE[key] = (plan, percore, nc)
    pf = _fold_params(params)
    in_maps = _make_inmaps(plan, percore, x, edge_attr, pf)
    res = bass_utils.run_bass_kernel_spmd(nc, in_maps, core_ids=list(range(NCORES)))
    ys = [res.results[c]['y'][0, :RPC] for c in range(NCORES)]
    return np.concatenate(ys)[:N_NODES, None].astype(np.float32)


# revision 2
# speedup vs baseline: 1.6210x; 1.6210x over previous
"""3-layer GAT (nn_GATModel) on 8 Trainium2 NeuronCores — self-contained kernel.

kernel(**inputs) takes the FULL inputs (x [50000,64], edge_index [2,800000],
edge_attr [800000,8], batch, params pytree) and returns the FULL [50000,1] output.

Sharding strategy (edge/graph partitioning per the hint, specialized):
  - Destination nodes are range-partitioned across the 8 cores (6250 real + 22 pad
    rows -> 6272 = 49*128 rows per core). Each core owns the complete segment
    softmax + aggregation for its destinations, so no per-edge collectives are
    needed; the only collective is one AllGather of the projected node table per
    layer.
  - Per layer, each core projects its own activations into table rows
    [xs | a_s | a_d | pad] (a_s/a_d/a_e are attention terms pre-folded into
    per-node scalars: att_src/att_dst/att_edge contract with W_src/W_dst/W_edge
    on the host into [d_in, H] matrices). AllGather -> full table on every core.
  - Edges are laid out as ELL slots per destination: 8 slots per src-half
    (src < 25088 vs >=, so gather indices fit int16 for the fast dma_gather
    ucode), padded with a dummy node whose a_s = -1e6 (=> exp weight exactly 0);
    extra edges go to one-hot overflow chunks appended to the same gather call.
  - Per (window of 128 dsts, half): ONE dma_gather fetches all slot rows
    (768B/512B rows), spread over 4 SWDGE queues for ~3x descriptor throughput.
    Attention scores are computed in place; one matmul per 128-slot chunk
    aggregates exp-weighted messages AND softmax denominators into PSUM
    (lhsT = static slot->dst one-hot; overflow chunks use is_equal-built one-hots).
  - Window epilogue: divide by denominator (+1e-16), add bias, PE-transpose to
    build the next layer's lhsT blocks; layer 3 applies the folded linear head
    (ll/fl collapse to [64,1]) and sigmoid, emitting the per-core output shard.
  - The segment softmax skips the max-subtraction (exactly equivalent
    mathematically; scores are O(10) so exp is safe in fp32), with the dummy
    slots underflowing to 0.
"""
import sys

for _p in ('/opt/trn_rl_repo', '/root/.axon_site/_ro/trn_rl_repo'):
    if _p not in sys.path:
        sys.path.insert(0, _p)

import numpy as np

import concourse.bass as bass
import concourse.bacc as bacc
import concourse.tile as tile
import concourse.mybir as mybir
from concourse import bass_utils
from concourse.library_config import mlp

P = 128
ELL = 8           # slots per dst per src-half
GD = 16           # dsts per chunk (= 128/ELL)
NCORES = 8

N_NODES = 50000
N_EDGES = 800000
RPC = N_NODES // NCORES          # 6250 real nodes per core
NW = (RPC + P - 1) // P          # 49 windows
NPC = NW * P                     # 6272 padded rows per core
N_PAD = NPC * NCORES             # 50176
HALF = NPC * (NCORES // 2)       # 25088 src split
DUMMY = [RPC, (NCORES // 2) * NPC + RPC]
WT = [192, 192, 128]             # table row stride per layer (256B multiples)
WX = [128, 128, 64]              # xs width per layer (H*C)
HA = [2, 2, 1]                   # heads per layer
WB = [132, 132, 66]              # written cols per layer


def _remap(ids):
    c = ids // RPC
    return c * NPC + (ids - c * RPC)


def _preprocess(edge_index):
    src = _remap(np.asarray(edge_index[0], np.int64))
    dst = _remap(np.asarray(edge_index[1], np.int64))
    core_of = dst // NPC
    halfb = (src >= HALF).astype(np.int64)

    novf = np.zeros((NCORES, NW, 2), np.int64)
    per_core_data = []
    for c in range(NCORES):
        m = core_of == c
        s_c, d_c, h_c, eid_c = src[m], dst[m] - c * NPC, halfb[m], np.nonzero(m)[0]
        w_c, dl_c = d_c // P, d_c % P
        order = np.lexsort((dl_c, h_c, w_c))
        s_c, dl_c, h_c, w_c, eid_c = (a[order] for a in (s_c, dl_c, h_c, w_c, eid_c))
        key = (w_c * 2 + h_c) * P + dl_c
        change = np.r_[True, key[1:] != key[:-1]]
        startidx = np.nonzero(change)[0]
        runlen = np.diff(np.r_[startidx, len(key)])
        rank = np.arange(len(key)) - np.repeat(startidx, runlen)
        ell_mask = rank < ELL
        per_core_data.append((s_c, dl_c, h_c, w_c, eid_c, rank, ell_mask))
        ov = ~ell_mask
        for w in range(NW):
            for hb in range(2):
                novf[c, w, hb] = np.sum(ov & (w_c == w) & (h_c == hb))

    ovch = np.zeros((NW, 2), np.int64)
    for w in range(NW):
        for hb in range(2):
            ovch[w, hb] = int((novf[:, w, hb].max() + P - 1) // P)

    S_wh = np.zeros((NW, 2), np.int64)
    offs = np.zeros((NW, 2), np.int64)
    ovf_col0 = np.zeros((NW, 2), np.int64)
    tot = 0
    ovftot = 0
    for w in range(NW):
        for hb in range(2):
            S_wh[w, hb] = ELL * P + ovch[w, hb] * P
            offs[w, hb] = tot
            tot += int(S_wh[w, hb])
            ovf_col0[w, hb] = ovftot
            ovftot += int(ovch[w, hb])
    plan = dict(S_wh=S_wh, offs=offs, ovch=ovch, ovf_col0=ovf_col0,
                tot_slots=tot, tot_ovf_chunks=ovftot)

    percore = []
    for c in range(NCORES):
        s_c, dl_c, h_c, w_c, eid_c, rank, em = per_core_data[c]
        slot_src = np.zeros(tot, np.int64)
        slot_eid = np.full(tot, -1, np.int64)
        ovf_dstloc = np.zeros((P, max(ovftot, 1)), np.float32)
        for w in range(NW):
            for hb in range(2):
                o = int(offs[w, hb])
                slot_src[o:o + int(S_wh[w, hb])] = DUMMY[hb]
        chunk = dl_c // GD
        pin = (dl_c % GD) * ELL + rank
        slot_pos = offs[w_c, h_c] + chunk * P + pin
        sp = slot_pos[em]
        slot_src[sp] = s_c[em]
        slot_eid[sp] = eid_c[em]
        ovm = ~em
        if np.any(ovm):
            for w in range(NW):
                for hb in range(2):
                    mm = ovm & (w_c == w) & (h_c == hb)
                    k = int(mm.sum())
                    if k == 0:
                        continue
                    base = int(offs[w, hb]) + ELL * P
                    pos = base + np.arange(k)
                    slot_src[pos] = s_c[mm]
                    slot_eid[pos] = eid_c[mm]
                    for j in range(int(ovch[w, hb])):
                        colv = np.zeros(P, np.float32)
                        lo, hi = j * P, min((j + 1) * P, k)
                        if lo < k:
                            colv[0:hi - lo] = dl_c[mm][lo:hi].astype(np.float32)
                        ovf_dstloc[:, int(ovf_col0[w, hb]) + j] = colv
        idx_rel = slot_src.copy()
        for w in range(NW):
            for hb in range(2):
                o = int(offs[w, hb])
                if hb == 1:
                    idx_rel[o:o + int(S_wh[w, hb])] -= HALF
        assert idx_rel.min() >= 0 and idx_rel.max() < 32768
        cols = tot // 16
        idx16 = np.zeros((16, cols), np.int16)
        pos = 0
        for w in range(NW):
            for hb in range(2):
                blk = idx_rel[int(offs[w, hb]):int(offs[w, hb]) + int(S_wh[w, hb])]
                n16 = len(blk) // 16
                idx16[:, pos:pos + n16] = blk.reshape(n16, 16).T
                pos += n16
        idx16 = np.tile(idx16, (8, 1))
        percore.append(dict(idx16=idx16, slot_eid=slot_eid, ovf_dstloc=ovf_dstloc))
    return plan, percore


def _fold_params(params):
    def g(p, k):
        return np.asarray(p[k], np.float32)
    out = {}
    for li in range(3):
        p = params[li]
        H, C = HA[li], 64
        W_src, W_dst, W_edge = g(p, 'W_src'), g(p, 'W_dst'), g(p, 'W_edge')
        a_s, a_d, a_e = g(p, 'att_src')[0], g(p, 'att_dst')[0], g(p, 'att_edge')[0]
        d_in = W_src.shape[0]
        As = np.zeros((d_in, H), np.float32)
        Ad = np.zeros((d_in, H), np.float32)
        V = np.zeros((8, H), np.float32)
        for h in range(H):
            As[:, h] = W_src[:, h * C:(h + 1) * C] @ a_s[h]
            Ad[:, h] = W_dst[:, h * C:(h + 1) * C] @ a_d[h]
            V[:, h] = W_edge[:, h * C:(h + 1) * C] @ a_e[h]
        out[f'rhsW{li}'] = np.concatenate([W_src, As, Ad], axis=1)
        out[f'V{li}'] = V
        out[f'bias{li}'] = np.broadcast_to(g(p, 'bias'), (P, H * C)).copy()
    hp = params[3]
    llW, llb = np.asarray(hp['ll_W'], np.float32), np.asarray(hp['ll_b'], np.float32)
    flW, flb = np.asarray(hp['fl_W'], np.float32), np.asarray(hp['fl_b'], np.float32)
    out['Whead'] = (llW @ flW).astype(np.float32)
    out['bhead'] = float(llb @ flW[:, 0] + flb[0])
    B128 = np.zeros((ELL, P, P), np.float32)
    for c in range(ELL):
        for p_ in range(P):
            B128[c, p_, GD * c + p_ // ELL] = 1.0
    out['B128'] = B128
    out['E128'] = np.ascontiguousarray(B128.transpose(0, 2, 1))
    import ml_dtypes
    out['B128b'] = B128.astype(ml_dtypes.bfloat16)
    out['E128b'] = out['E128'].astype(ml_dtypes.bfloat16)
    out['identb'] = np.eye(P, dtype=ml_dtypes.bfloat16)
    for li in range(3):
        out[f'Vb{li}'] = out[f'V{li}'].astype(ml_dtypes.bfloat16)
    out['iota'] = np.broadcast_to(np.arange(P, dtype=np.float32), (P, P)).copy()
    out['ident'] = np.eye(P, dtype=np.float32)
    return out


def _build_nc(plan):
    nc = bacc.Bacc("TRN2", target_bir_lowering=False, debug=False,
                   num_devices=NCORES, num_swdge_queues=4)
    dt = mybir.dt.float32
    S_wh, offs, ovch, ovf_col0 = plan['S_wh'], plan['offs'], plan['ovch'], plan['ovf_col0']
    tot, totov = plan['tot_slots'], max(plan['tot_ovf_chunks'], 1)

    xT0 = nc.dram_tensor("xT0", [64, NPC], dt, kind="ExternalInput")
    idx16 = nc.dram_tensor("idx16", [P, tot // 16], mybir.dt.int16, kind="ExternalInput")
    eaT = nc.dram_tensor("eaT", [8, tot], mybir.dt.bfloat16, kind="ExternalInput")
    ovfdl = nc.dram_tensor("ovfdl", [P, totov], dt, kind="ExternalInput")
    prm = {}
    for li in range(3):
        d_in = 64 if li == 0 else 128
        prm[f'rhsW{li}'] = nc.dram_tensor(f"rhsW{li}", [d_in, WB[li]], dt, kind="ExternalInput")
        prm[f'V{li}'] = nc.dram_tensor(f"V{li}", [8, HA[li]], mybir.dt.bfloat16, kind="ExternalInput")
        prm[f'bias{li}'] = nc.dram_tensor(f"bias{li}", [P, WX[li]], dt, kind="ExternalInput")
    prm['Whead'] = nc.dram_tensor("Whead", [64, 1], dt, kind="ExternalInput")
    prm['bheadneg'] = nc.dram_tensor("bheadneg", [1, 1], dt, kind="ExternalInput")
    prm['B128'] = nc.dram_tensor("B128", [ELL, P, P], dt, kind="ExternalInput")
    prm['padmask'] = nc.dram_tensor("padmask", [P, 2], dt, kind="ExternalInput")
    prm['E128'] = nc.dram_tensor("E128", [ELL, P, P], dt, kind="ExternalInput")
    prm['B128b'] = nc.dram_tensor("B128b", [ELL, P, P], mybir.dt.bfloat16, kind="ExternalInput")
    prm['E128b'] = nc.dram_tensor("E128b", [ELL, P, P], mybir.dt.bfloat16, kind="ExternalInput")
    prm['identb'] = nc.dram_tensor("identb", [P, P], mybir.dt.bfloat16, kind="ExternalInput")
    prm['iota'] = nc.dram_tensor("iota", [P, P], dt, kind="ExternalInput")
    prm['ident'] = nc.dram_tensor("ident", [P, P], dt, kind="ExternalInput")
    y_out = nc.dram_tensor("y", [1, NPC], dt, kind="ExternalOutput")

    xT = [None,
          nc.dram_tensor("xT1", [P, NPC], dt, kind="Internal"),
          nc.dram_tensor("xT2", [P, NPC], dt, kind="Internal")]
    Tin, Tfull = [], []
    for li in range(3):
        Tin.append(nc.dram_tensor(f"Tin{li}", [NPC, WT[li]], dt, kind="Internal"))
        Tfull.append(nc.dram_tensor(f"Tfull{li}", [N_PAD, WT[li]], dt,
                                    kind="Internal", addr_space="Shared"))

    rr = [0]
    with tile.TileContext(nc) as tc:
        with (
            tc.tile_pool(name="const", bufs=1) as cp,
            tc.tile_pool(name="sbuf", bufs=3) as sb,
            tc.tile_pool(name="gpool", bufs=3) as gp,
            tc.tile_pool(name="psum", bufs=2, space="PSUM") as ps,
            tc.tile_pool(name="psmisc", bufs=2, space="PSUM") as pm,
        ):
            nc.gpsimd.load_library(mlp)
            c_t = {}
            for name, shape in [('padmask', [P, 2]), ('iota', [P, P]),
                                ('ident', [P, P]), ('Whead', [64, 1]), ('bheadneg', [1, 1])]:
                c_t[name] = cp.tile(shape, dt, name=name, tag=name)
                nc.sync.dma_start(out=c_t[name][:], in_=prm[name][:])
            bt = mybir.dt.bfloat16
            for c in range(ELL):
                for nm in ('B128b', 'E128b'):
                    c_t[f'{nm}_{c}'] = cp.tile([P, P], bt, name=f'{nm}_{c}', tag=f'{nm}_{c}')
                    nc.sync.dma_start(out=c_t[f'{nm}_{c}'][:], in_=prm[nm][c])
            c_t['identb'] = cp.tile([P, P], bt, name='identb', tag='identb')
            nc.sync.dma_start(out=c_t['identb'][:], in_=prm['identb'][:])
            for li in range(3):
                d_in = 64 if li == 0 else 128
                c_t[f'rhsW{li}'] = cp.tile([d_in, WB[li]], dt, name=f"rhsW{li}", tag=f"rhsW{li}")
                nc.sync.dma_start(out=c_t[f'rhsW{li}'][:], in_=prm[f'rhsW{li}'][:])
                c_t[f'V{li}'] = cp.tile([8, HA[li]], mybir.dt.bfloat16, name=f"V{li}", tag=f"V{li}")
                nc.sync.dma_start(out=c_t[f'V{li}'][:], in_=prm[f'V{li}'][:])
                c_t[f'bias{li}'] = cp.tile([P, WX[li]], dt, name=f"bias{li}", tag=f"bias{li}")
                nc.sync.dma_start(out=c_t[f'bias{li}'][:], in_=prm[f'bias{li}'][:])

            for li in range(3):
                WTl, WXl, HAl, WBl = WT[li], WX[li], HA[li], WB[li]
                d_in = 64 if li == 0 else 128
                for j in range(NW):
                    lhs = sb.tile([d_in, P], dt, tag="tb_lhs")
                    src_ap = xT0[:, j * P:(j + 1) * P] if li == 0 else xT[li][:, j * P:(j + 1) * P]
                    nc.sync.dma_start(out=lhs[:], in_=src_ap)
                    pt = pm.tile([P, WBl], dt, space="PSUM", tag="tb_ps", bufs=1)
                    nc.tensor.matmul(pt[:], lhsT=lhs[:], rhs=c_t[f'rhsW{li}'][:],
                                     start=True, stop=True)
                    ts = sb.tile([P, WBl], dt, tag="tb_sb")
                    nc.vector.tensor_copy(ts[:], pt[:])
                    if j == NW - 1:
                        nc.vector.tensor_tensor(
                            out=ts[:, WXl:WXl + HAl], in0=ts[:, WXl:WXl + HAl],
                            in1=c_t['padmask'][:, 0:HAl], op=mybir.AluOpType.add)
                    nc.sync.dma_start(out=Tin[li][j * P:(j + 1) * P, 0:WBl], in_=ts[:])
                nc.gpsimd.collective_compute(
                    "AllGather", mybir.AluOpType.bypass,
                    replica_groups=[list(range(NCORES))],
                    ins=[Tin[li][:, :]], outs=[Tfull[li][:, :]],
                )
                for w in range(NW):
                    adw = sb.tile([P, HAl], dt, tag="adw")
                    nc.sync.dma_start(
                        out=adw[:],
                        in_=Tin[li][w * P:(w + 1) * P, WXl + HAl:WXl + 2 * HAl],
                    )
                    adwb = sb.tile([P, HAl], mybir.dt.bfloat16, tag="adwb")
                    nc.vector.tensor_copy(adwb[:], adw[:])
                    pw = ps.tile([P, WXl + HAl], dt, space="PSUM", tag="pwin")
                    half_data = []
                    for hb in range(2):
                        S = int(S_wh[w, hb])
                        K = S // P
                        o = int(offs[w, hb])
                        idx_t = sb.tile([P, S // 16], mybir.dt.int16, tag="idx")
                        nc.sync.dma_start(out=idx_t[:], in_=idx16[:, o // 16:(o + S) // 16])
                        ea_t = sb.tile([8, S], mybir.dt.bfloat16, tag="ea")
                        nc.sync.dma_start(out=ea_t[:], in_=eaT[:, o:o + S])
                        g = gp.tile([P, K, WTl], dt, tag=f"g{hb}")
                        base = Tfull[li][HALF:, 0:WTl] if hb else Tfull[li][:, 0:WTl]
                        nc.gpsimd.dma_gather(
                            out_ap=g[:], in_ap=base, idxs_ap=idx_t[:],
                            num_idxs=S, num_idxs_reg=S, elem_size=WTl, elem_step=WTl,
                            single_packet=False, queue_num=rr[0] % 4,
                        )
                        rr[0] += 1
                        pae = ps.tile([P, K * HAl], dt, space="PSUM", tag=f"pae{hb}", bufs=1)
                        ovP = []
                        for k in range(K):
                            nc.tensor.matmul(
                                pae[:, k * HAl:(k + 1) * HAl],
                                lhsT=ea_t[:, k * P:(k + 1) * P], rhs=c_t[f'V{li}'][:],
                                start=True, stop=False)
                            if k < ELL:
                                nc.tensor.matmul(
                                    pae[:, k * HAl:(k + 1) * HAl],
                                    lhsT=c_t[f'E128b_{k}'][:], rhs=adwb[:, :],
                                    start=False, stop=True)
                            else:
                                oc = int(ovf_col0[w, hb]) + (k - ELL)
                                p_sb = sb.tile([P, P], mybir.dt.bfloat16, tag=f"povf{hb}_{k - ELL}")
                                dl = sb.tile([P, 1], dt, tag=f"dl{hb}_{k - ELL}")
                                nc.sync.dma_start(out=dl[:], in_=ovfdl[:, oc:oc + 1])
                                nc.vector.tensor_tensor(
                                    out=p_sb[:], in0=dl[:].to_broadcast([P, P]),
                                    in1=c_t['iota'][:], op=mybir.AluOpType.is_equal)
                                ptp = pm.tile([P, P], mybir.dt.bfloat16, space="PSUM", tag="ptransb", bufs=1)
                                nc.tensor.transpose(out=ptp[:], in_=p_sb[:], identity=c_t['identb'][:])
                                pT_sb = sb.tile([P, P], mybir.dt.bfloat16, tag=f"pT{hb}_{k - ELL}")
                                nc.vector.tensor_copy(pT_sb[:], ptp[:])
                                nc.tensor.matmul(
                                    pae[:, k * HAl:(k + 1) * HAl],
                                    lhsT=pT_sb[:], rhs=adwb[:, :],
                                    start=False, stop=True)
                                ovP.append(p_sb)
                        t_a = sb.tile([P, K, HAl], dt, tag=f"ta{hb}")
                        nc.vector.tensor_tensor(
                            out=t_a[:], in0=g[:, :, WXl:WXl + HAl],
                            in1=pae[:].rearrange("p (k h) -> p k h", h=HAl),
                            op=mybir.AluOpType.add)
                        t2 = sb.tile([P, K, HAl], dt, tag=f"t2{hb}")
                        nc.vector.tensor_scalar(out=t2[:], in0=t_a[:], scalar1=0.2,
                                                scalar2=None, op0=mybir.AluOpType.mult)
                        lr = sb.tile([P, K, HAl], dt, tag=f"lr{hb}")
                        nc.vector.tensor_tensor(out=lr[:], in0=t_a[:], in1=t2[:],
                                                op=mybir.AluOpType.max)
                        nc.scalar.activation(g[:, :, WXl:WXl + HAl], lr[:],
                                             mybir.ActivationFunctionType.Exp)
                        gb = gp.tile([P, K, WXl + HAl], mybir.dt.bfloat16, tag=f"gb{hb}")
                        for h in range(HAl):
                            nc.vector.tensor_tensor(
                                out=gb[:, :, h * 64:(h + 1) * 64],
                                in0=g[:, :, h * 64:(h + 1) * 64],
                                in1=g[:, :, WXl + h:WXl + h + 1].to_broadcast([P, K, 64]),
                                op=mybir.AluOpType.mult)
                        nc.vector.tensor_copy(gb[:, :, WXl:WXl + HAl], g[:, :, WXl:WXl + HAl])
                        half_data.append((gb, ovP, K))
                    ktot = half_data[0][2] + half_data[1][2]
                    kc = 0
                    for hb in range(2):
                        g, ovP, K = half_data[hb]
                        for k in range(K):
                            lh = c_t[f'B128b_{k}'][:] if k < ELL else ovP[k - ELL][:]
                            nc.tensor.matmul(
                                pw[:, :], lhsT=lh, rhs=g[:, k:k + 1, 0:WXl + HAl],
                                start=(kc == 0), stop=(kc == ktot - 1),
                                skip_group_check=True)
                            kc += 1
                    dn = sb.tile([P, HAl], dt, tag="dn")
                    nc.vector.tensor_scalar(out=dn[:], in0=pw[:, WXl:WXl + HAl],
                                            scalar1=1e-16, scalar2=None,
                                            op0=mybir.AluOpType.add)
                    rc = sb.tile([P, HAl], dt, tag="rc")
                    nc.vector.reciprocal(rc[:], dn[:])
                    o_sb = sb.tile([P, WXl], dt, tag="osb")
                    for h in range(HAl):
                        nc.vector.tensor_scalar(
                            out=o_sb[:, h * 64:(h + 1) * 64], in0=pw[:, h * 64:(h + 1) * 64],
                            scalar1=rc[:, h:h + 1], scalar2=None, op0=mybir.AluOpType.mult)
                    nc.vector.tensor_tensor(out=o_sb[:], in0=o_sb[:], in1=c_t[f'bias{li}'][:],
                                            op=mybir.AluOpType.add)
                    ptp2 = pm.tile([WXl, P], dt, space="PSUM", tag="ptrans", bufs=1)
                    nc.tensor.transpose(out=ptp2[:], in_=o_sb[:], identity=c_t['ident'][:])
                    if li < 2:
                        xts = sb.tile([WXl, P], dt, tag="xts")
                        nc.vector.tensor_copy(xts[:], ptp2[:])
                        nc.sync.dma_start(out=xT[li + 1][:, w * P:(w + 1) * P], in_=xts[:])
                    else:
                        o3t = sb.tile([64, P], dt, tag="o3t")
                        nc.vector.tensor_copy(o3t[:], ptp2[:])
                        py = pm.tile([1, P], dt, space="PSUM", tag="py", bufs=1)
                        nc.tensor.matmul(py[:], lhsT=c_t['Whead'][:], rhs=o3t[:],
                                         start=True, stop=True)
                        es = sb.tile([1, P], dt, tag="es")
                        nc.scalar.activation(es[:], py[:], mybir.ActivationFunctionType.Exp,
                                             scale=-1.0, bias=c_t['bheadneg'][0:1, 0:1])
                        e1 = sb.tile([1, P], dt, tag="e1")
                        nc.vector.tensor_scalar(out=e1[:], in0=es[:], scalar1=1.0,
                                                scalar2=None, op0=mybir.AluOpType.add)
                        ys = sb.tile([1, P], dt, tag="ys")
                        nc.vector.reciprocal(ys[:], e1[:])
                        nc.sync.dma_start(out=y_out[:, w * P:(w + 1) * P], in_=ys[:])
    nc.compile()
    return nc


def _make_inmaps(plan, percore, x, edge_attr, pf):
    x = np.asarray(x, np.float32)
    ea = np.asarray(edge_attr, np.float32)
    in_maps = []
    pmsk = np.zeros((P, 2), np.float32)
    if RPC % P:
        pmsk[RPC % P:, :] = -1e6
    for c in range(NCORES):
        d = percore[c]
        xT0 = np.zeros((64, NPC), np.float32)
        xs = x[c * RPC:(c + 1) * RPC]
        xT0[:, :xs.shape[0]] = xs.T
        import ml_dtypes
        eaT = np.zeros((8, plan['tot_slots']), ml_dtypes.bfloat16)
        m = d['slot_eid'] >= 0
        eaT[:, m] = ea[d['slot_eid'][m]].T.astype(ml_dtypes.bfloat16)
        im = dict(xT0=xT0, idx16=d['idx16'], eaT=eaT, ovfdl=d['ovf_dstloc'],
                  padmask=pmsk)
        for li in range(3):
            im[f'rhsW{li}'] = pf[f'rhsW{li}']
            im[f'V{li}'] = pf[f'V{li}']
            im[f'bias{li}'] = pf[f'bias{li}']
        im['Whead'] = pf['Whead']
        im['bheadneg'] = np.array([[-pf['bhead']]], np.float32)
        for k in ('B128', 'E128', 'B128b', 'E128b', 'iota', 'ident', 'identb'):
            im[k] = pf[k]
        for li in range(3):
            im[f'V{li}'] = pf[f'Vb{li}']
        in_maps.append(im)
    return in_maps


_CACHE = {}


def kernel(x, edge_index, edge_attr, batch=None, params=None):
    ei = np.asarray(edge_index).astype(np.int64)
    key = hash(ei.tobytes())
    if key in _CACHE:
        plan, percore, nc = _CACHE[key]
    else:
        plan, percore = _preprocess(ei)
        nc = _build_nc(plan)
        _CACHE[key] = (plan, percore, nc)
    pf = _fold_params(params)
    in_maps = _make_inmaps(plan, percore, x, edge_attr, pf)
    res = bass_utils.run_bass_kernel_spmd(nc, in_maps, core_ids=list(range(NCORES)))
    ys = [res.results[c]['y'][0, :RPC] for c in range(NCORES)]
    return np.concatenate(ys)[:N_NODES, None].astype(np.float32)


# revision 3
# speedup vs baseline: 1.6302x; 1.0057x over previous
"""3-layer GAT (nn_GATModel) on 8 Trainium2 NeuronCores — self-contained kernel.

kernel(**inputs) takes the FULL inputs (x [50000,64], edge_index [2,800000],
edge_attr [800000,8], batch, params pytree) and returns the FULL [50000,1] output.

Sharding strategy (edge/graph partitioning per the hint, specialized):
  - Destination nodes are range-partitioned across the 8 cores (6250 real + 22 pad
    rows -> 6272 = 49*128 rows per core). Each core owns the complete segment
    softmax + aggregation for its destinations, so no per-edge collectives are
    needed; the only collective is one AllGather of the projected node table per
    layer.
  - Per layer, each core projects its own activations into table rows
    [xs | a_s | a_d | pad] (a_s/a_d/a_e are attention terms pre-folded into
    per-node scalars: att_src/att_dst/att_edge contract with W_src/W_dst/W_edge
    on the host into [d_in, H] matrices). AllGather -> full table on every core.
  - Edges are laid out as ELL slots per destination: 8 slots per src-half
    (src < 25088 vs >=, so gather indices fit int16 for the fast dma_gather
    ucode), padded with a dummy node whose a_s = -1e6 (=> exp weight exactly 0);
    extra edges go to one-hot overflow chunks appended to the same gather call.
  - Per (window of 128 dsts, half): ONE dma_gather fetches all slot rows
    (768B/512B rows), spread over 4 SWDGE queues for ~3x descriptor throughput.
    Attention scores are computed in place; one matmul per 128-slot chunk
    aggregates exp-weighted messages AND softmax denominators into PSUM
    (lhsT = static slot->dst one-hot; overflow chunks use is_equal-built one-hots).
  - Window epilogue: divide by denominator (+1e-16), add bias, PE-transpose to
    build the next layer's lhsT blocks; layer 3 applies the folded linear head
    (ll/fl collapse to [64,1]) and sigmoid, emitting the per-core output shard.
  - The segment softmax skips the max-subtraction (exactly equivalent
    mathematically; scores are O(10) so exp is safe in fp32), with the dummy
    slots underflowing to 0.
"""
import sys

for _p in ('/opt/trn_rl_repo', '/root/.axon_site/_ro/trn_rl_repo'):
    if _p not in sys.path:
        sys.path.insert(0, _p)

import numpy as np

import concourse.bass as bass
import concourse.bacc as bacc
import concourse.tile as tile
import concourse.mybir as mybir
from concourse import bass_utils
from concourse.library_config import mlp

P = 128
ELL = 8           # slots per dst per src-half
GD = 16           # dsts per chunk (= 128/ELL)
NCORES = 8

N_NODES = 50000
N_EDGES = 800000
RPC = N_NODES // NCORES          # 6250 real nodes per core
NW = (RPC + P - 1) // P          # 49 windows
NPC = NW * P                     # 6272 padded rows per core
N_PAD = NPC * NCORES             # 50176
HALF = NPC * (NCORES // 2)       # 25088 src split
DUMMY = [RPC, (NCORES // 2) * NPC + RPC]
WT = [192, 192, 128]             # table row stride per layer (256B multiples)
WX = [128, 128, 64]              # xs width per layer (H*C)
HA = [2, 2, 1]                   # heads per layer
WB = [132, 132, 66]              # written cols per layer


def _remap(ids):
    c = ids // RPC
    return c * NPC + (ids - c * RPC)


def _preprocess(edge_index):
    src = _remap(np.asarray(edge_index[0], np.int64))
    dst = _remap(np.asarray(edge_index[1], np.int64))
    core_of = dst // NPC
    halfb = (src >= HALF).astype(np.int64)

    novf = np.zeros((NCORES, NW, 2), np.int64)
    per_core_data = []
    for c in range(NCORES):
        m = core_of == c
        s_c, d_c, h_c, eid_c = src[m], dst[m] - c * NPC, halfb[m], np.nonzero(m)[0]
        w_c, dl_c = d_c // P, d_c % P
        order = np.lexsort((dl_c, h_c, w_c))
        s_c, dl_c, h_c, w_c, eid_c = (a[order] for a in (s_c, dl_c, h_c, w_c, eid_c))
        key = (w_c * 2 + h_c) * P + dl_c
        change = np.r_[True, key[1:] != key[:-1]]
        startidx = np.nonzero(change)[0]
        runlen = np.diff(np.r_[startidx, len(key)])
        rank = np.arange(len(key)) - np.repeat(startidx, runlen)
        ell_mask = rank < ELL
        per_core_data.append((s_c, dl_c, h_c, w_c, eid_c, rank, ell_mask))
        ov = ~ell_mask
        for w in range(NW):
            for hb in range(2):
                novf[c, w, hb] = np.sum(ov & (w_c == w) & (h_c == hb))

    ovch = np.zeros((NW, 2), np.int64)
    for w in range(NW):
        for hb in range(2):
            ovch[w, hb] = int((novf[:, w, hb].max() + P - 1) // P)

    S_wh = np.zeros((NW, 2), np.int64)
    offs = np.zeros((NW, 2), np.int64)
    ovf_col0 = np.zeros((NW, 2), np.int64)
    tot = 0
    ovftot = 0
    for w in range(NW):
        for hb in range(2):
            S_wh[w, hb] = ELL * P + ovch[w, hb] * P
            offs[w, hb] = tot
            tot += int(S_wh[w, hb])
            ovf_col0[w, hb] = ovftot
            ovftot += int(ovch[w, hb])
    plan = dict(S_wh=S_wh, offs=offs, ovch=ovch, ovf_col0=ovf_col0,
                tot_slots=tot, tot_ovf_chunks=ovftot)

    percore = []
    for c in range(NCORES):
        s_c, dl_c, h_c, w_c, eid_c, rank, em = per_core_data[c]
        slot_src = np.zeros(tot, np.int64)
        slot_eid = np.full(tot, -1, np.int64)
        ovf_dstloc = np.zeros((P, max(ovftot, 1)), np.float32)
        for w in range(NW):
            for hb in range(2):
                o = int(offs[w, hb])
                slot_src[o:o + int(S_wh[w, hb])] = DUMMY[hb]
        chunk = dl_c // GD
        pin = (dl_c % GD) * ELL + rank
        slot_pos = offs[w_c, h_c] + chunk * P + pin
        sp = slot_pos[em]
        slot_src[sp] = s_c[em]
        slot_eid[sp] = eid_c[em]
        ovm = ~em
        if np.any(ovm):
            for w in range(NW):
                for hb in range(2):
                    mm = ovm & (w_c == w) & (h_c == hb)
                    k = int(mm.sum())
                    if k == 0:
                        continue
                    base = int(offs[w, hb]) + ELL * P
                    pos = base + np.arange(k)
                    slot_src[pos] = s_c[mm]
                    slot_eid[pos] = eid_c[mm]
                    for j in range(int(ovch[w, hb])):
                        colv = np.zeros(P, np.float32)
                        lo, hi = j * P, min((j + 1) * P, k)
                        if lo < k:
                            colv[0:hi - lo] = dl_c[mm][lo:hi].astype(np.float32)
                        ovf_dstloc[:, int(ovf_col0[w, hb]) + j] = colv
        idx_rel = slot_src.copy()
        for w in range(NW):
            for hb in range(2):
                o = int(offs[w, hb])
                if hb == 1:
                    idx_rel[o:o + int(S_wh[w, hb])] -= HALF
        assert idx_rel.min() >= 0 and idx_rel.max() < 32768
        cols = tot // 16
        idx16 = np.zeros((16, cols), np.int16)
        pos = 0
        for w in range(NW):
            for hb in range(2):
                blk = idx_rel[int(offs[w, hb]):int(offs[w, hb]) + int(S_wh[w, hb])]
                n16 = len(blk) // 16
                idx16[:, pos:pos + n16] = blk.reshape(n16, 16).T
                pos += n16
        idx16 = np.tile(idx16, (8, 1))
        percore.append(dict(idx16=idx16, slot_eid=slot_eid, ovf_dstloc=ovf_dstloc))
    return plan, percore


def _fold_params(params):
    def g(p, k):
        return np.asarray(p[k], np.float32)
    out = {}
    for li in range(3):
        p = params[li]
        H, C = HA[li], 64
        W_src, W_dst, W_edge = g(p, 'W_src'), g(p, 'W_dst'), g(p, 'W_edge')
        a_s, a_d, a_e = g(p, 'att_src')[0], g(p, 'att_dst')[0], g(p, 'att_edge')[0]
        d_in = W_src.shape[0]
        As = np.zeros((d_in, H), np.float32)
        Ad = np.zeros((d_in, H), np.float32)
        V = np.zeros((8, H), np.float32)
        for h in range(H):
            As[:, h] = W_src[:, h * C:(h + 1) * C] @ a_s[h]
            Ad[:, h] = W_dst[:, h * C:(h + 1) * C] @ a_d[h]
            V[:, h] = W_edge[:, h * C:(h + 1) * C] @ a_e[h]
        out[f'rhsW{li}'] = np.concatenate([W_src, As, Ad], axis=1)
        out[f'V{li}'] = V
        out[f'bias{li}'] = np.broadcast_to(g(p, 'bias'), (P, H * C)).copy()
    hp = params[3]
    llW, llb = np.asarray(hp['ll_W'], np.float32), np.asarray(hp['ll_b'], np.float32)
    flW, flb = np.asarray(hp['fl_W'], np.float32), np.asarray(hp['fl_b'], np.float32)
    out['Whead'] = (llW @ flW).astype(np.float32)
    out['bhead'] = float(llb @ flW[:, 0] + flb[0])
    B128 = np.zeros((ELL, P, P), np.float32)
    for c in range(ELL):
        for p_ in range(P):
            B128[c, p_, GD * c + p_ // ELL] = 1.0
    out['B128'] = B128
    out['E128'] = np.ascontiguousarray(B128.transpose(0, 2, 1))
    import ml_dtypes
    out['B128b'] = B128.astype(ml_dtypes.bfloat16)
    out['E128b'] = out['E128'].astype(ml_dtypes.bfloat16)
    out['identb'] = np.eye(P, dtype=ml_dtypes.bfloat16)
    for li in range(3):
        out[f'Vb{li}'] = out[f'V{li}'].astype(ml_dtypes.bfloat16)
    out['iota'] = np.broadcast_to(np.arange(P, dtype=np.float32), (P, P)).copy()
    out['ident'] = np.eye(P, dtype=np.float32)
    return out


def _build_nc(plan):
    nc = bacc.Bacc("TRN2", target_bir_lowering=False, debug=False,
                   num_devices=NCORES, num_swdge_queues=4)
    dt = mybir.dt.float32
    S_wh, offs, ovch, ovf_col0 = plan['S_wh'], plan['offs'], plan['ovch'], plan['ovf_col0']
    tot, totov = plan['tot_slots'], max(plan['tot_ovf_chunks'], 1)

    xT0 = nc.dram_tensor("xT0", [64, NPC], dt, kind="ExternalInput")
    idx16 = nc.dram_tensor("idx16", [P, tot // 16], mybir.dt.int16, kind="ExternalInput")
    eaT = nc.dram_tensor("eaT", [8, tot], mybir.dt.bfloat16, kind="ExternalInput")
    ovfdl = nc.dram_tensor("ovfdl", [P, totov], dt, kind="ExternalInput")
    prm = {}
    for li in range(3):
        d_in = 64 if li == 0 else 128
        prm[f'rhsW{li}'] = nc.dram_tensor(f"rhsW{li}", [d_in, WB[li]], dt, kind="ExternalInput")
        prm[f'V{li}'] = nc.dram_tensor(f"V{li}", [8, HA[li]], mybir.dt.bfloat16, kind="ExternalInput")
        prm[f'bias{li}'] = nc.dram_tensor(f"bias{li}", [P, WX[li]], dt, kind="ExternalInput")
    prm['Whead'] = nc.dram_tensor("Whead", [64, 1], dt, kind="ExternalInput")
    prm['bheadneg'] = nc.dram_tensor("bheadneg", [1, 1], dt, kind="ExternalInput")
    prm['B128'] = nc.dram_tensor("B128", [ELL, P, P], dt, kind="ExternalInput")
    prm['padmask'] = nc.dram_tensor("padmask", [P, 2], dt, kind="ExternalInput")
    prm['E128'] = nc.dram_tensor("E128", [ELL, P, P], dt, kind="ExternalInput")
    prm['B128b'] = nc.dram_tensor("B128b", [ELL, P, P], mybir.dt.bfloat16, kind="ExternalInput")
    prm['E128b'] = nc.dram_tensor("E128b", [ELL, P, P], mybir.dt.bfloat16, kind="ExternalInput")
    prm['identb'] = nc.dram_tensor("identb", [P, P], mybir.dt.bfloat16, kind="ExternalInput")
    prm['iota'] = nc.dram_tensor("iota", [P, P], dt, kind="ExternalInput")
    prm['ident'] = nc.dram_tensor("ident", [P, P], dt, kind="ExternalInput")
    y_out = nc.dram_tensor("y", [1, NPC], dt, kind="ExternalOutput")

    xT = [None,
          nc.dram_tensor("xT1", [P, NPC], dt, kind="Internal"),
          nc.dram_tensor("xT2", [P, NPC], dt, kind="Internal")]
    Tin, Tfull = [], []
    for li in range(3):
        Tin.append(nc.dram_tensor(f"Tin{li}", [NPC, WT[li]], dt, kind="Internal"))
        Tfull.append(nc.dram_tensor(f"Tfull{li}", [N_PAD, WT[li]], dt,
                                    kind="Internal", addr_space="Shared"))

    rr = [0]
    with tile.TileContext(nc) as tc:
        with (
            tc.tile_pool(name="const", bufs=1) as cp,
            tc.tile_pool(name="sbuf", bufs=3) as sb,
            tc.tile_pool(name="gpool", bufs=3) as gp,
            tc.tile_pool(name="psum", bufs=2, space="PSUM") as ps,
            tc.tile_pool(name="psmisc", bufs=2, space="PSUM") as pm,
        ):
            nc.gpsimd.load_library(mlp)
            c_t = {}
            for name, shape in [('padmask', [P, 2]), ('iota', [P, P]),
                                ('ident', [P, P]), ('Whead', [64, 1]), ('bheadneg', [1, 1])]:
                c_t[name] = cp.tile(shape, dt, name=name, tag=name)
                nc.sync.dma_start(out=c_t[name][:], in_=prm[name][:])
            bt = mybir.dt.bfloat16
            for c in range(ELL):
                for nm in ('B128b', 'E128b'):
                    c_t[f'{nm}_{c}'] = cp.tile([P, P], bt, name=f'{nm}_{c}', tag=f'{nm}_{c}')
                    nc.sync.dma_start(out=c_t[f'{nm}_{c}'][:], in_=prm[nm][c])
            c_t['identb'] = cp.tile([P, P], bt, name='identb', tag='identb')
            nc.sync.dma_start(out=c_t['identb'][:], in_=prm['identb'][:])
            for li in range(3):
                d_in = 64 if li == 0 else 128
                c_t[f'rhsW{li}'] = cp.tile([d_in, WB[li]], dt, name=f"rhsW{li}", tag=f"rhsW{li}")
                nc.sync.dma_start(out=c_t[f'rhsW{li}'][:], in_=prm[f'rhsW{li}'][:])
                c_t[f'V{li}'] = cp.tile([8, HA[li]], mybir.dt.bfloat16, name=f"V{li}", tag=f"V{li}")
                nc.sync.dma_start(out=c_t[f'V{li}'][:], in_=prm[f'V{li}'][:])
                c_t[f'bias{li}'] = cp.tile([P, WX[li]], dt, name=f"bias{li}", tag=f"bias{li}")
                nc.sync.dma_start(out=c_t[f'bias{li}'][:], in_=prm[f'bias{li}'][:])

            for li in range(3):
                WTl, WXl, HAl, WBl = WT[li], WX[li], HA[li], WB[li]
                d_in = 64 if li == 0 else 128
                for j in range(NW):
                    lhs = sb.tile([d_in, P], dt, tag="tb_lhs")
                    src_ap = xT0[:, j * P:(j + 1) * P] if li == 0 else xT[li][:, j * P:(j + 1) * P]
                    nc.sync.dma_start(out=lhs[:], in_=src_ap)
                    pt = pm.tile([P, WBl], dt, space="PSUM", tag="tb_ps", bufs=1)
                    nc.tensor.matmul(pt[:], lhsT=lhs[:], rhs=c_t[f'rhsW{li}'][:],
                                     start=True, stop=True)
                    ts = sb.tile([P, WBl], dt, tag="tb_sb")
                    nc.vector.tensor_copy(ts[:], pt[:])
                    if j == NW - 1:
                        nc.vector.tensor_tensor(
                            out=ts[:, WXl:WXl + HAl], in0=ts[:, WXl:WXl + HAl],
                            in1=c_t['padmask'][:, 0:HAl], op=mybir.AluOpType.add)
                    nc.sync.dma_start(out=Tin[li][j * P:(j + 1) * P, 0:WBl], in_=ts[:])
                nc.gpsimd.collective_compute(
                    "AllGather", mybir.AluOpType.bypass,
                    replica_groups=[list(range(NCORES))],
                    ins=[Tin[li][:, :]], outs=[Tfull[li][:, :]],
                )
                for w in range(NW):
                    adw = sb.tile([P, HAl], dt, tag="adw")
                    nc.sync.dma_start(
                        out=adw[:],
                        in_=Tin[li][w * P:(w + 1) * P, WXl + HAl:WXl + 2 * HAl],
                    )
                    adwb = sb.tile([P, HAl], mybir.dt.bfloat16, tag="adwb")
                    nc.vector.tensor_copy(adwb[:], adw[:])
                    pw = ps.tile([P, WXl + HAl], dt, space="PSUM", tag="pwin")
                    half_data = []
                    for hb in range(2):
                        S = int(S_wh[w, hb])
                        K = S // P
                        o = int(offs[w, hb])
                        idx_t = sb.tile([P, S // 16], mybir.dt.int16, tag="idx")
                        nc.sync.dma_start(out=idx_t[:], in_=idx16[:, o // 16:(o + S) // 16])
                        ea_t = sb.tile([8, S], mybir.dt.bfloat16, tag="ea")
                        nc.sync.dma_start(out=ea_t[:], in_=eaT[:, o:o + S])
                        g = gp.tile([P, K, WTl], dt, tag=f"g{hb}")
                        base = Tfull[li][HALF:, 0:WTl] if hb else Tfull[li][:, 0:WTl]
                        nc.gpsimd.dma_gather(
                            out_ap=g[:], in_ap=base, idxs_ap=idx_t[:],
                            num_idxs=S, num_idxs_reg=S, elem_size=WTl, elem_step=WTl,
                            single_packet=False, queue_num=rr[0] % 4,
                        )
                        rr[0] += 1
                        pae = ps.tile([P, K * HAl], dt, space="PSUM", tag=f"pae{hb}", bufs=1)
                        ovP = []
                        for k in range(K):
                            nc.tensor.matmul(
                                pae[:, k * HAl:(k + 1) * HAl],
                                lhsT=ea_t[:, k * P:(k + 1) * P], rhs=c_t[f'V{li}'][:],
                                start=True, stop=False)
                            if k < ELL:
                                nc.tensor.matmul(
                                    pae[:, k * HAl:(k + 1) * HAl],
                                    lhsT=c_t[f'E128b_{k}'][:], rhs=adwb[:, :],
                                    start=False, stop=True)
                            else:
                                oc = int(ovf_col0[w, hb]) + (k - ELL)
                                p_sb = sb.tile([P, P], mybir.dt.bfloat16, tag=f"povf{hb}_{k - ELL}")
                                dl = sb.tile([P, 1], dt, tag=f"dl{hb}_{k - ELL}")
                                nc.sync.dma_start(out=dl[:], in_=ovfdl[:, oc:oc + 1])
                                nc.vector.tensor_tensor(
                                    out=p_sb[:], in0=dl[:].to_broadcast([P, P]),
                                    in1=c_t['iota'][:], op=mybir.AluOpType.is_equal)
                                ptp = pm.tile([P, P], mybir.dt.bfloat16, space="PSUM", tag="ptransb", bufs=1)
                                nc.tensor.transpose(out=ptp[:], in_=p_sb[:], identity=c_t['identb'][:])
                                pT_sb = sb.tile([P, P], mybir.dt.bfloat16, tag=f"pT{hb}_{k - ELL}")
                                nc.vector.tensor_copy(pT_sb[:], ptp[:])
                                nc.tensor.matmul(
                                    pae[:, k * HAl:(k + 1) * HAl],
                                    lhsT=pT_sb[:], rhs=adwb[:, :],
                                    start=False, stop=True)
                                ovP.append(p_sb)
                        t_a = sb.tile([P, K, HAl], dt, tag=f"ta{hb}")
                        nc.vector.tensor_tensor(
                            out=t_a[:], in0=g[:, :, WXl:WXl + HAl],
                            in1=pae[:].rearrange("p (k h) -> p k h", h=HAl),
                            op=mybir.AluOpType.add)
                        t2 = sb.tile([P, K, HAl], dt, tag=f"t2{hb}")
                        nc.vector.tensor_scalar(out=t2[:], in0=t_a[:], scalar1=0.2,
                                                scalar2=None, op0=mybir.AluOpType.mult)
                        lr = sb.tile([P, K, HAl], dt, tag=f"lr{hb}")
                        nc.vector.tensor_tensor(out=lr[:], in0=t_a[:], in1=t2[:],
                                                op=mybir.AluOpType.max)
                        nc.scalar.activation(g[:, :, WXl:WXl + HAl], lr[:],
                                             mybir.ActivationFunctionType.Exp)
                        gb = gp.tile([P, K, WXl + HAl], mybir.dt.bfloat16, tag=f"gb{hb}")
                        for h in range(HAl):
                            nc.vector.tensor_tensor(
                                out=gb[:, :, h * 64:(h + 1) * 64],
                                in0=g[:, :, h * 64:(h + 1) * 64],
                                in1=g[:, :, WXl + h:WXl + h + 1].to_broadcast([P, K, 64]),
                                op=mybir.AluOpType.mult)
                        nc.vector.tensor_copy(gb[:, :, WXl:WXl + HAl], g[:, :, WXl:WXl + HAl])
                        half_data.append((gb, ovP, K))
                    ktot = half_data[0][2] + half_data[1][2]
                    kc = 0
                    for hb in range(2):
                        g, ovP, K = half_data[hb]
                        for k in range(K):
                            lh = c_t[f'B128b_{k}'][:] if k < ELL else ovP[k - ELL][:]
                            nc.tensor.matmul(
                                pw[:, :], lhsT=lh, rhs=g[:, k:k + 1, 0:WXl + HAl],
                                start=(kc == 0), stop=(kc == ktot - 1),
                                skip_group_check=True)
                            kc += 1
                    dn = sb.tile([P, HAl], dt, tag="dn")
                    nc.vector.tensor_scalar(out=dn[:], in0=pw[:, WXl:WXl + HAl],
                                            scalar1=1e-16, scalar2=None,
                                            op0=mybir.AluOpType.add)
                    rc = sb.tile([P, HAl], dt, tag="rc")
                    nc.vector.reciprocal(rc[:], dn[:])
                    o_sb = sb.tile([P, WXl], dt, tag="osb")
                    for h in range(HAl):
                        nc.vector.tensor_scalar(
                            out=o_sb[:, h * 64:(h + 1) * 64], in0=pw[:, h * 64:(h + 1) * 64],
                            scalar1=rc[:, h:h + 1], scalar2=None, op0=mybir.AluOpType.mult)
                    nc.vector.tensor_tensor(out=o_sb[:], in0=o_sb[:], in1=c_t[f'bias{li}'][:],
                                            op=mybir.AluOpType.add)
                    ptp2 = pm.tile([WXl, P], dt, space="PSUM", tag="ptrans", bufs=1)
                    nc.tensor.transpose(out=ptp2[:], in_=o_sb[:], identity=c_t['ident'][:])
                    if li < 2:
                        xts = sb.tile([WXl, P], dt, tag="xts")
                        nc.vector.tensor_copy(xts[:], ptp2[:])
                        nc.sync.dma_start(out=xT[li + 1][:, w * P:(w + 1) * P], in_=xts[:])
                    else:
                        o3t = sb.tile([64, P], dt, tag="o3t")
                        nc.vector.tensor_copy(o3t[:], ptp2[:])
                        py = pm.tile([1, P], dt, space="PSUM", tag="py", bufs=1)
                        nc.tensor.matmul(py[:], lhsT=c_t['Whead'][:], rhs=o3t[:],
                                         start=True, stop=True)
                        es = sb.tile([1, P], dt, tag="es")
                        nc.scalar.activation(es[:], py[:], mybir.ActivationFunctionType.Exp,
                                             scale=-1.0, bias=c_t['bheadneg'][0:1, 0:1])
                        e1 = sb.tile([1, P], dt, tag="e1")
                        nc.vector.tensor_scalar(out=e1[:], in0=es[:], scalar1=1.0,
                                                scalar2=None, op0=mybir.AluOpType.add)
                        ys = sb.tile([1, P], dt, tag="ys")
                        nc.vector.reciprocal(ys[:], e1[:])
                        nc.sync.dma_start(out=y_out[:, w * P:(w + 1) * P], in_=ys[:])
    nc.compile()
    return nc


def _make_inmaps(plan, percore, x, edge_attr, pf):
    x = np.asarray(x, np.float32)
    ea = np.asarray(edge_attr, np.float32)
    in_maps = []
    pmsk = np.zeros((P, 2), np.float32)
    if RPC % P:
        pmsk[RPC % P:, :] = -1e6
    for c in range(NCORES):
        d = percore[c]
        xT0 = np.zeros((64, NPC), np.float32)
        xs = x[c * RPC:(c + 1) * RPC]
        xT0[:, :xs.shape[0]] = xs.T
        import ml_dtypes
        eaT = np.zeros((8, plan['tot_slots']), ml_dtypes.bfloat16)
        m = d['slot_eid'] >= 0
        eaT[:, m] = ea[d['slot_eid'][m]].T.astype(ml_dtypes.bfloat16)
        im = dict(xT0=xT0, idx16=d['idx16'], eaT=eaT, ovfdl=d['ovf_dstloc'],
                  padmask=pmsk)
        for li in range(3):
            im[f'rhsW{li}'] = pf[f'rhsW{li}']
            im[f'V{li}'] = pf[f'V{li}']
            im[f'bias{li}'] = pf[f'bias{li}']
        im['Whead'] = pf['Whead']
        im['bheadneg'] = np.array([[-pf['bhead']]], np.float32)
        for k in ('B128', 'E128', 'B128b', 'E128b', 'iota', 'ident', 'identb'):
            im[k] = pf[k]
        for li in range(3):
            im[f'V{li}'] = pf[f'Vb{li}']
        in_maps.append(im)
    return in_maps


_CACHE = {}


def kernel(x, edge_index, edge_attr, batch=None, params=None):
    ei = np.asarray(edge_index).astype(np.int64)
    key = hash(ei.tobytes())
    if key in _CACHE:
        plan, percore, nc = _CACHE[key]
    else:
        plan, percore = _preprocess(ei)
        nc = _build_nc(plan)
        _CACHE[key] = (plan, percore, nc)
    pf = _fold_params(params)
    in_maps = _make_inmaps(plan, percore, x, edge_attr, pf)
    import time as _time
    res = None
    for attempt in range(3):
        try:
            res = bass_utils.run_bass_kernel_spmd(nc, in_maps, core_ids=list(range(NCORES)))
            break
        except Exception:
            if attempt == 2:
                raise
            _time.sleep(3.0)
    assert res is not None
    ys = [res.results[c]['y'][0, :RPC] for c in range(NCORES)]
    return np.concatenate(ys)[:N_NODES, None].astype(np.float32)


# revision 4
# speedup vs baseline: 1.6857x; 1.0340x over previous
"""3-layer GAT (nn_GATModel) on 8 Trainium2 NeuronCores — self-contained kernel.

kernel(**inputs) takes the FULL inputs (x [50000,64], edge_index [2,800000],
edge_attr [800000,8], batch, params pytree) and returns the FULL [50000,1] output.

Sharding strategy (edge/graph partitioning per the hint, specialized):
  - Destination nodes are range-partitioned across the 8 cores (6250 real + 22 pad
    rows -> 6272 = 49*128 rows per core). Each core owns the complete segment
    softmax + aggregation for its destinations, so no per-edge collectives are
    needed; the only collective is one AllGather of the projected node table per
    layer.
  - Per layer, each core projects its own activations into table rows
    [xs | a_s | a_d | pad] (a_s/a_d/a_e are attention terms pre-folded into
    per-node scalars: att_src/att_dst/att_edge contract with W_src/W_dst/W_edge
    on the host into [d_in, H] matrices). AllGather -> full table on every core.
  - Edges are laid out as ELL slots per destination: 8 slots per src-half
    (src < 25088 vs >=, so gather indices fit int16 for the fast dma_gather
    ucode), padded with a dummy node whose a_s = -1e6 (=> exp weight exactly 0);
    extra edges go to one-hot overflow chunks appended to the same gather call.
  - Per (window of 128 dsts, half): ONE dma_gather fetches all slot rows
    (768B/512B rows), spread over 4 SWDGE queues for ~3x descriptor throughput.
    Attention scores are computed in place; one matmul per 128-slot chunk
    aggregates exp-weighted messages AND softmax denominators into PSUM
    (lhsT = static slot->dst one-hot; overflow chunks use is_equal-built one-hots).
  - Window epilogue: divide by denominator (+1e-16), add bias, PE-transpose to
    build the next layer's lhsT blocks; layer 3 applies the folded linear head
    (ll/fl collapse to [64,1]) and sigmoid, emitting the per-core output shard.
  - The segment softmax skips the max-subtraction (exactly equivalent
    mathematically; scores are O(10) so exp is safe in fp32), with the dummy
    slots underflowing to 0.
"""
import sys

for _p in ('/opt/trn_rl_repo', '/root/.axon_site/_ro/trn_rl_repo'):
    if _p not in sys.path:
        sys.path.insert(0, _p)

import numpy as np

import concourse.bass as bass
import concourse.bacc as bacc
import concourse.tile as tile
import concourse.mybir as mybir
from concourse import bass_utils
from concourse.library_config import mlp

P = 128
ELL = 8           # slots per dst per src-half
GD = 16           # dsts per chunk (= 128/ELL)
NCORES = 8

N_NODES = 50000
N_EDGES = 800000
RPC = N_NODES // NCORES          # 6250 real nodes per core
NW = (RPC + P - 1) // P          # 49 windows
NPC = NW * P                     # 6272 padded rows per core
N_PAD = NPC * NCORES             # 50176
HALF = NPC * (NCORES // 2)       # 25088 src split
DUMMY = [RPC, (NCORES // 2) * NPC + RPC]
WT = [192, 192, 128]             # table row stride per layer (256B multiples)
WX = [128, 128, 64]              # xs width per layer (H*C)
HA = [2, 2, 1]                   # heads per layer
WB = [132, 132, 66]              # written cols per layer


def _remap(ids):
    c = ids // RPC
    return c * NPC + (ids - c * RPC)


def _preprocess(edge_index):
    src = _remap(np.asarray(edge_index[0], np.int64))
    dst = _remap(np.asarray(edge_index[1], np.int64))
    core_of = dst // NPC
    halfb = (src >= HALF).astype(np.int64)

    novf = np.zeros((NCORES, NW, 2), np.int64)
    per_core_data = []
    for c in range(NCORES):
        m = core_of == c
        s_c, d_c, h_c, eid_c = src[m], dst[m] - c * NPC, halfb[m], np.nonzero(m)[0]
        w_c, dl_c = d_c // P, d_c % P
        order = np.lexsort((dl_c, h_c, w_c))
        s_c, dl_c, h_c, w_c, eid_c = (a[order] for a in (s_c, dl_c, h_c, w_c, eid_c))
        key = (w_c * 2 + h_c) * P + dl_c
        change = np.r_[True, key[1:] != key[:-1]]
        startidx = np.nonzero(change)[0]
        runlen = np.diff(np.r_[startidx, len(key)])
        rank = np.arange(len(key)) - np.repeat(startidx, runlen)
        ell_mask = rank < ELL
        per_core_data.append((s_c, dl_c, h_c, w_c, eid_c, rank, ell_mask))
        ov = ~ell_mask
        for w in range(NW):
            for hb in range(2):
                novf[c, w, hb] = np.sum(ov & (w_c == w) & (h_c == hb))

    ovch = np.zeros((NW, 2), np.int64)
    for w in range(NW):
        for hb in range(2):
            ovch[w, hb] = int((novf[:, w, hb].max() + P - 1) // P)

    S_wh = np.zeros((NW, 2), np.int64)
    offs = np.zeros((NW, 2), np.int64)
    ovf_col0 = np.zeros((NW, 2), np.int64)
    tot = 0
    ovftot = 0
    for w in range(NW):
        for hb in range(2):
            S_wh[w, hb] = ELL * P + ovch[w, hb] * P
            offs[w, hb] = tot
            tot += int(S_wh[w, hb])
            ovf_col0[w, hb] = ovftot
            ovftot += int(ovch[w, hb])
    plan = dict(S_wh=S_wh, offs=offs, ovch=ovch, ovf_col0=ovf_col0,
                tot_slots=tot, tot_ovf_chunks=ovftot)

    percore = []
    for c in range(NCORES):
        s_c, dl_c, h_c, w_c, eid_c, rank, em = per_core_data[c]
        slot_src = np.zeros(tot, np.int64)
        slot_eid = np.full(tot, -1, np.int64)
        ovf_dstloc = np.zeros((P, max(ovftot, 1)), np.float32)
        for w in range(NW):
            for hb in range(2):
                o = int(offs[w, hb])
                slot_src[o:o + int(S_wh[w, hb])] = DUMMY[hb]
        chunk = dl_c // GD
        pin = (dl_c % GD) * ELL + rank
        slot_pos = offs[w_c, h_c] + chunk * P + pin
        sp = slot_pos[em]
        slot_src[sp] = s_c[em]
        slot_eid[sp] = eid_c[em]
        ovm = ~em
        if np.any(ovm):
            for w in range(NW):
                for hb in range(2):
                    mm = ovm & (w_c == w) & (h_c == hb)
                    k = int(mm.sum())
                    if k == 0:
                        continue
                    base = int(offs[w, hb]) + ELL * P
                    pos = base + np.arange(k)
                    slot_src[pos] = s_c[mm]
                    slot_eid[pos] = eid_c[mm]
                    for j in range(int(ovch[w, hb])):
                        colv = np.zeros(P, np.float32)
                        lo, hi = j * P, min((j + 1) * P, k)
                        if lo < k:
                            colv[0:hi - lo] = dl_c[mm][lo:hi].astype(np.float32)
                        ovf_dstloc[:, int(ovf_col0[w, hb]) + j] = colv
        idx_rel = slot_src.copy()
        for w in range(NW):
            for hb in range(2):
                o = int(offs[w, hb])
                if hb == 1:
                    idx_rel[o:o + int(S_wh[w, hb])] -= HALF
        assert idx_rel.min() >= 0 and idx_rel.max() < 32768
        cols = tot // 16
        idx16 = np.zeros((16, cols), np.int16)
        pos = 0
        for w in range(NW):
            for hb in range(2):
                blk = idx_rel[int(offs[w, hb]):int(offs[w, hb]) + int(S_wh[w, hb])]
                n16 = len(blk) // 16
                idx16[:, pos:pos + n16] = blk.reshape(n16, 16).T
                pos += n16
        idx16 = np.tile(idx16, (8, 1))
        percore.append(dict(idx16=idx16, slot_eid=slot_eid, ovf_dstloc=ovf_dstloc))
    return plan, percore


def _fold_params(params):
    def g(p, k):
        return np.asarray(p[k], np.float32)
    out = {}
    for li in range(3):
        p = params[li]
        H, C = HA[li], 64
        W_src, W_dst, W_edge = g(p, 'W_src'), g(p, 'W_dst'), g(p, 'W_edge')
        a_s, a_d, a_e = g(p, 'att_src')[0], g(p, 'att_dst')[0], g(p, 'att_edge')[0]
        d_in = W_src.shape[0]
        As = np.zeros((d_in, H), np.float32)
        Ad = np.zeros((d_in, H), np.float32)
        V = np.zeros((8, H), np.float32)
        for h in range(H):
            As[:, h] = W_src[:, h * C:(h + 1) * C] @ a_s[h]
            Ad[:, h] = W_dst[:, h * C:(h + 1) * C] @ a_d[h]
            V[:, h] = W_edge[:, h * C:(h + 1) * C] @ a_e[h]
        out[f'rhsW{li}'] = np.concatenate([W_src, As, Ad], axis=1)
        out[f'V{li}'] = V
        out[f'bias{li}'] = np.broadcast_to(g(p, 'bias'), (P, H * C)).copy()
    hp = params[3]
    llW, llb = np.asarray(hp['ll_W'], np.float32), np.asarray(hp['ll_b'], np.float32)
    flW, flb = np.asarray(hp['fl_W'], np.float32), np.asarray(hp['fl_b'], np.float32)
    out['Whead'] = (llW @ flW).astype(np.float32)
    out['bhead'] = float(llb @ flW[:, 0] + flb[0])
    B128 = np.zeros((ELL, P, P), np.float32)
    for c in range(ELL):
        for p_ in range(P):
            B128[c, p_, GD * c + p_ // ELL] = 1.0
    out['B128'] = B128
    out['E128'] = np.ascontiguousarray(B128.transpose(0, 2, 1))
    import ml_dtypes
    out['B128b'] = B128.astype(ml_dtypes.bfloat16)
    out['E128b'] = out['E128'].astype(ml_dtypes.bfloat16)
    out['identb'] = np.eye(P, dtype=ml_dtypes.bfloat16)
    for li in range(3):
        out[f'Vb{li}'] = out[f'V{li}'].astype(ml_dtypes.bfloat16)
    out['iota'] = np.broadcast_to(np.arange(P, dtype=np.float32), (P, P)).copy()
    out['ident'] = np.eye(P, dtype=np.float32)
    return out


def _build_nc(plan):
    nc = bacc.Bacc("TRN2", target_bir_lowering=False, debug=False,
                   num_devices=NCORES, num_swdge_queues=4)
    dt = mybir.dt.float32
    S_wh, offs, ovch, ovf_col0 = plan['S_wh'], plan['offs'], plan['ovch'], plan['ovf_col0']
    tot, totov = plan['tot_slots'], max(plan['tot_ovf_chunks'], 1)

    xT0 = nc.dram_tensor("xT0", [64, NPC], dt, kind="ExternalInput")
    idx16 = nc.dram_tensor("idx16", [P, tot // 16], mybir.dt.int16, kind="ExternalInput")
    eaT = nc.dram_tensor("eaT", [8, tot], mybir.dt.bfloat16, kind="ExternalInput")
    ovfdl = nc.dram_tensor("ovfdl", [P, totov], dt, kind="ExternalInput")
    prm = {}
    for li in range(3):
        d_in = 64 if li == 0 else 128
        prm[f'rhsW{li}'] = nc.dram_tensor(f"rhsW{li}", [d_in, WB[li]], dt, kind="ExternalInput")
        prm[f'V{li}'] = nc.dram_tensor(f"V{li}", [8, HA[li]], mybir.dt.bfloat16, kind="ExternalInput")
        prm[f'bias{li}'] = nc.dram_tensor(f"bias{li}", [P, WX[li]], dt, kind="ExternalInput")
    prm['Whead'] = nc.dram_tensor("Whead", [64, 1], dt, kind="ExternalInput")
    prm['bheadneg'] = nc.dram_tensor("bheadneg", [1, 1], dt, kind="ExternalInput")
    prm['B128'] = nc.dram_tensor("B128", [ELL, P, P], dt, kind="ExternalInput")
    prm['padmask'] = nc.dram_tensor("padmask", [P, 2], dt, kind="ExternalInput")
    prm['E128'] = nc.dram_tensor("E128", [ELL, P, P], dt, kind="ExternalInput")
    prm['B128b'] = nc.dram_tensor("B128b", [ELL, P, P], mybir.dt.bfloat16, kind="ExternalInput")
    prm['E128b'] = nc.dram_tensor("E128b", [ELL, P, P], mybir.dt.bfloat16, kind="ExternalInput")
    prm['identb'] = nc.dram_tensor("identb", [P, P], mybir.dt.bfloat16, kind="ExternalInput")
    prm['iota'] = nc.dram_tensor("iota", [P, P], dt, kind="ExternalInput")
    prm['ident'] = nc.dram_tensor("ident", [P, P], dt, kind="ExternalInput")
    y_out = nc.dram_tensor("y", [1, NPC], dt, kind="ExternalOutput")

    xT = [None,
          nc.dram_tensor("xT1", [P, NPC], dt, kind="Internal"),
          nc.dram_tensor("xT2", [P, NPC], dt, kind="Internal")]
    Tin, Tfull = [], []
    for li in range(3):
        Tin.append(nc.dram_tensor(f"Tin{li}", [NPC, WT[li]], dt, kind="Internal"))
        Tfull.append(nc.dram_tensor(f"Tfull{li}", [N_PAD, WT[li]], dt,
                                    kind="Internal", addr_space="Shared"))

    rr = [0]
    with tile.TileContext(nc) as tc:
        with (
            tc.tile_pool(name="const", bufs=1) as cp,
            tc.tile_pool(name="sbuf", bufs=4) as sb,
            tc.tile_pool(name="gpool", bufs=4) as gp,
            tc.tile_pool(name="psum", bufs=2, space="PSUM") as ps,
            tc.tile_pool(name="psmisc", bufs=2, space="PSUM") as pm,
        ):
            nc.gpsimd.load_library(mlp)
            c_t = {}
            for name, shape in [('padmask', [P, 2]), ('iota', [P, P]),
                                ('ident', [P, P]), ('Whead', [64, 1]), ('bheadneg', [1, 1])]:
                c_t[name] = cp.tile(shape, dt, name=name, tag=name)
                nc.sync.dma_start(out=c_t[name][:], in_=prm[name][:])
            bt = mybir.dt.bfloat16
            for c in range(ELL):
                for nm in ('B128b', 'E128b'):
                    c_t[f'{nm}_{c}'] = cp.tile([P, P], bt, name=f'{nm}_{c}', tag=f'{nm}_{c}')
                    nc.sync.dma_start(out=c_t[f'{nm}_{c}'][:], in_=prm[nm][c])
            c_t['identb'] = cp.tile([P, P], bt, name='identb', tag='identb')
            nc.sync.dma_start(out=c_t['identb'][:], in_=prm['identb'][:])
            for li in range(3):
                d_in = 64 if li == 0 else 128
                c_t[f'rhsW{li}'] = cp.tile([d_in, WB[li]], dt, name=f"rhsW{li}", tag=f"rhsW{li}")
                nc.sync.dma_start(out=c_t[f'rhsW{li}'][:], in_=prm[f'rhsW{li}'][:])
                c_t[f'V{li}'] = cp.tile([8, HA[li]], mybir.dt.bfloat16, name=f"V{li}", tag=f"V{li}")
                nc.sync.dma_start(out=c_t[f'V{li}'][:], in_=prm[f'V{li}'][:])
                c_t[f'bias{li}'] = cp.tile([P, WX[li]], dt, name=f"bias{li}", tag=f"bias{li}")
                nc.sync.dma_start(out=c_t[f'bias{li}'][:], in_=prm[f'bias{li}'][:])

            for li in range(3):
                WTl, WXl, HAl, WBl = WT[li], WX[li], HA[li], WB[li]
                d_in = 64 if li == 0 else 128
                for j in range(NW):
                    lhs = sb.tile([d_in, P], dt, tag="tb_lhs")
                    src_ap = xT0[:, j * P:(j + 1) * P] if li == 0 else xT[li][:, j * P:(j + 1) * P]
                    nc.sync.dma_start(out=lhs[:], in_=src_ap)
                    pt = pm.tile([P, WBl], dt, space="PSUM", tag="ptrans", bufs=1)
                    nc.tensor.matmul(pt[:], lhsT=lhs[:], rhs=c_t[f'rhsW{li}'][:],
                                     start=True, stop=True)
                    ts = sb.tile([P, WBl], dt, tag="tb_sb")
                    nc.vector.tensor_copy(ts[:], pt[:])
                    if j == NW - 1:
                        nc.vector.tensor_tensor(
                            out=ts[:, WXl:WXl + HAl], in0=ts[:, WXl:WXl + HAl],
                            in1=c_t['padmask'][:, 0:HAl], op=mybir.AluOpType.add)
                    nc.sync.dma_start(out=Tin[li][j * P:(j + 1) * P, 0:WBl], in_=ts[:])
                nc.gpsimd.collective_compute(
                    "AllGather", mybir.AluOpType.bypass,
                    replica_groups=[list(range(NCORES))],
                    ins=[Tin[li][:, :]], outs=[Tfull[li][:, :]],
                )
                for w in range(NW):
                    adw = sb.tile([P, HAl], dt, tag="adw")
                    nc.sync.dma_start(
                        out=adw[:],
                        in_=Tin[li][w * P:(w + 1) * P, WXl + HAl:WXl + 2 * HAl],
                    )
                    adwb = sb.tile([P, HAl], mybir.dt.bfloat16, tag="adwb")
                    nc.vector.tensor_copy(adwb[:], adw[:])
                    pw = ps.tile([P, WXl + HAl], dt, space="PSUM", tag="pwin")
                    half_data = []
                    for hb in range(2):
                        S = int(S_wh[w, hb])
                        K = S // P
                        o = int(offs[w, hb])
                        idx_t = sb.tile([P, S // 16], mybir.dt.int16, tag="idx")
                        nc.sync.dma_start(out=idx_t[:], in_=idx16[:, o // 16:(o + S) // 16])
                        ea_t = sb.tile([8, S], mybir.dt.bfloat16, tag="ea")
                        nc.sync.dma_start(out=ea_t[:], in_=eaT[:, o:o + S])
                        g = gp.tile([P, K, WTl], dt, tag=f"g{hb}")
                        base = Tfull[li][HALF:, 0:WTl] if hb else Tfull[li][:, 0:WTl]
                        nc.gpsimd.dma_gather(
                            out_ap=g[:], in_ap=base, idxs_ap=idx_t[:],
                            num_idxs=S, num_idxs_reg=S, elem_size=WTl, elem_step=WTl,
                            single_packet=False, queue_num=rr[0] % 4,
                        )
                        rr[0] += 1
                        pae = ps.tile([P, K * HAl], dt, space="PSUM", tag=f"pae{hb}", bufs=2)
                        ovP = []
                        for k in range(K):
                            nc.tensor.matmul(
                                pae[:, k * HAl:(k + 1) * HAl],
                                lhsT=ea_t[:, k * P:(k + 1) * P], rhs=c_t[f'V{li}'][:],
                                start=True, stop=False)
                            if k < ELL:
                                nc.tensor.matmul(
                                    pae[:, k * HAl:(k + 1) * HAl],
                                    lhsT=c_t[f'E128b_{k}'][:], rhs=adwb[:, :],
                                    start=False, stop=True)
                            else:
                                oc = int(ovf_col0[w, hb]) + (k - ELL)
                                p_sb = sb.tile([P, P], mybir.dt.bfloat16, tag=f"povf{hb}_{k - ELL}")
                                dl = sb.tile([P, 1], dt, tag=f"dl{hb}_{k - ELL}")
                                nc.sync.dma_start(out=dl[:], in_=ovfdl[:, oc:oc + 1])
                                nc.vector.tensor_tensor(
                                    out=p_sb[:], in0=dl[:].to_broadcast([P, P]),
                                    in1=c_t['iota'][:], op=mybir.AluOpType.is_equal)
                                ptp = pm.tile([P, P], mybir.dt.bfloat16, space="PSUM", tag="ptransb", bufs=1)
                                nc.tensor.transpose(out=ptp[:], in_=p_sb[:], identity=c_t['identb'][:])
                                pT_sb = sb.tile([P, P], mybir.dt.bfloat16, tag=f"pT{hb}_{k - ELL}")
                                nc.vector.tensor_copy(pT_sb[:], ptp[:])
                                nc.tensor.matmul(
                                    pae[:, k * HAl:(k + 1) * HAl],
                                    lhsT=pT_sb[:], rhs=adwb[:, :],
                                    start=False, stop=True)
                                ovP.append(p_sb)
                        t_a = sb.tile([P, K, HAl], dt, tag=f"ta{hb}")
                        nc.vector.tensor_tensor(
                            out=t_a[:], in0=g[:, :, WXl:WXl + HAl],
                            in1=pae[:].rearrange("p (k h) -> p k h", h=HAl),
                            op=mybir.AluOpType.add)
                        t2 = sb.tile([P, K, HAl], dt, tag=f"t2{hb}")
                        nc.vector.tensor_scalar(out=t2[:], in0=t_a[:], scalar1=0.2,
                                                scalar2=None, op0=mybir.AluOpType.mult)
                        lr = sb.tile([P, K, HAl], dt, tag=f"lr{hb}")
                        nc.vector.tensor_tensor(out=lr[:], in0=t_a[:], in1=t2[:],
                                                op=mybir.AluOpType.max)
                        nc.scalar.activation(g[:, :, WXl:WXl + HAl], lr[:],
                                             mybir.ActivationFunctionType.Exp)
                        gb = gp.tile([P, K, WXl + HAl], mybir.dt.bfloat16, tag=f"gb{hb}")
                        for h in range(HAl):
                            nc.vector.tensor_tensor(
                                out=gb[:, :, h * 64:(h + 1) * 64],
                                in0=g[:, :, h * 64:(h + 1) * 64],
                                in1=g[:, :, WXl + h:WXl + h + 1].to_broadcast([P, K, 64]),
                                op=mybir.AluOpType.mult)
                        nc.vector.tensor_copy(gb[:, :, WXl:WXl + HAl], g[:, :, WXl:WXl + HAl])
                        half_data.append((gb, ovP, K))
                    ktot = half_data[0][2] + half_data[1][2]
                    kc = 0
                    for hb in range(2):
                        g, ovP, K = half_data[hb]
                        for k in range(K):
                            lh = c_t[f'B128b_{k}'][:] if k < ELL else ovP[k - ELL][:]
                            nc.tensor.matmul(
                                pw[:, :], lhsT=lh, rhs=g[:, k:k + 1, 0:WXl + HAl],
                                start=(kc == 0), stop=(kc == ktot - 1),
                                skip_group_check=True)
                            kc += 1
                    dn = sb.tile([P, HAl], dt, tag="dn")
                    nc.vector.tensor_scalar(out=dn[:], in0=pw[:, WXl:WXl + HAl],
                                            scalar1=1e-16, scalar2=None,
                                            op0=mybir.AluOpType.add)
                    rc = sb.tile([P, HAl], dt, tag="rc")
                    nc.vector.reciprocal(rc[:], dn[:])
                    o_sb = sb.tile([P, WXl], dt, tag="osb")
                    for h in range(HAl):
                        nc.vector.tensor_scalar(
                            out=o_sb[:, h * 64:(h + 1) * 64], in0=pw[:, h * 64:(h + 1) * 64],
                            scalar1=rc[:, h:h + 1], scalar2=None, op0=mybir.AluOpType.mult)
                    nc.vector.tensor_tensor(out=o_sb[:], in0=o_sb[:], in1=c_t[f'bias{li}'][:],
                                            op=mybir.AluOpType.add)
                    ptp2 = pm.tile([WXl, P], dt, space="PSUM", tag="ptrans", bufs=1)
                    nc.tensor.transpose(out=ptp2[:], in_=o_sb[:], identity=c_t['ident'][:])
                    if li < 2:
                        xts = sb.tile([WXl, P], dt, tag="xts")
                        nc.vector.tensor_copy(xts[:], ptp2[:])
                        nc.sync.dma_start(out=xT[li + 1][:, w * P:(w + 1) * P], in_=xts[:])
                    else:
                        o3t = sb.tile([64, P], dt, tag="o3t")
                        nc.vector.tensor_copy(o3t[:], ptp2[:])
                        py = pm.tile([1, P], dt, space="PSUM", tag="ptrans", bufs=1)
                        nc.tensor.matmul(py[:], lhsT=c_t['Whead'][:], rhs=o3t[:],
                                         start=True, stop=True)
                        es = sb.tile([1, P], dt, tag="es")
                        nc.scalar.activation(es[:], py[:], mybir.ActivationFunctionType.Exp,
                                             scale=-1.0, bias=c_t['bheadneg'][0:1, 0:1])
                        e1 = sb.tile([1, P], dt, tag="e1")
                        nc.vector.tensor_scalar(out=e1[:], in0=es[:], scalar1=1.0,
                                                scalar2=None, op0=mybir.AluOpType.add)
                        ys = sb.tile([1, P], dt, tag="ys")
                        nc.vector.reciprocal(ys[:], e1[:])
                        nc.sync.dma_start(out=y_out[:, w * P:(w + 1) * P], in_=ys[:])
    nc.compile()
    return nc


def _make_inmaps(plan, percore, x, edge_attr, pf):
    x = np.asarray(x, np.float32)
    ea = np.asarray(edge_attr, np.float32)
    in_maps = []
    pmsk = np.zeros((P, 2), np.float32)
    if RPC % P:
        pmsk[RPC % P:, :] = -1e6
    for c in range(NCORES):
        d = percore[c]
        xT0 = np.zeros((64, NPC), np.float32)
        xs = x[c * RPC:(c + 1) * RPC]
        xT0[:, :xs.shape[0]] = xs.T
        import ml_dtypes
        eaT = np.zeros((8, plan['tot_slots']), ml_dtypes.bfloat16)
        m = d['slot_eid'] >= 0
        eaT[:, m] = ea[d['slot_eid'][m]].T.astype(ml_dtypes.bfloat16)
        im = dict(xT0=xT0, idx16=d['idx16'], eaT=eaT, ovfdl=d['ovf_dstloc'],
                  padmask=pmsk)
        for li in range(3):
            im[f'rhsW{li}'] = pf[f'rhsW{li}']
            im[f'V{li}'] = pf[f'V{li}']
            im[f'bias{li}'] = pf[f'bias{li}']
        im['Whead'] = pf['Whead']
        im['bheadneg'] = np.array([[-pf['bhead']]], np.float32)
        for k in ('B128', 'E128', 'B128b', 'E128b', 'iota', 'ident', 'identb'):
            im[k] = pf[k]
        for li in range(3):
            im[f'V{li}'] = pf[f'Vb{li}']
        in_maps.append(im)
    return in_maps


_CACHE = {}


def kernel(x, edge_index, edge_attr, batch=None, params=None):
    ei = np.asarray(edge_index).astype(np.int64)
    key = hash(ei.tobytes())
    if key in _CACHE:
        plan, percore, nc = _CACHE[key]
    else:
        plan, percore = _preprocess(ei)
        nc = _build_nc(plan)
        _CACHE[key] = (plan, percore, nc)
    pf = _fold_params(params)
    in_maps = _make_inmaps(plan, percore, x, edge_attr, pf)
    import time as _time
    res = None
    for attempt in range(3):
        try:
            res = bass_utils.run_bass_kernel_spmd(nc, in_maps, core_ids=list(range(NCORES)))
            break
        except Exception:
            if attempt == 2:
                raise
            _time.sleep(3.0)
    assert res is not None
    ys = [res.results[c]['y'][0, :RPC] for c in range(NCORES)]
    return np.concatenate(ys)[:N_NODES, None].astype(np.float32)
